# revision 19
# baseline (speedup 1.0000x reference)
"""Trainium2 Bass kernel for nn_BNet (hypergraph GNN message passing), 8 cores.

V2 design (vs baseline):
- Phase A has NO device gather: h[node]|pin_feature is staged host-side per
  pin slot (hpf, bf16, hyperedge-major degree-grouped layout, the same class
  of index-driven staging the baseline uses for pin_feature/aidx). One strided
  DVE reduce per run gives per-hyperedge sums; e_feat+v come from one PE
  transpose + one matmul per tile against R = [[W1],[Wpin],[b1] | .@att2]
  built on device. This removes the xl table, its AllGather, and 1310 of the
  baseline's 2396 serialized INDIRECT1D instructions.
- ef table is bf16 66-wide (64 e_feat + v + pad), so the phase C gather moves
  132B/row instead of 260B, and the AllGather shrinks 2x.
- Phase C keeps per-column indirect_dma_start (a 122k-row table cannot use
  the batched int16 dma_gather), ~1086 instructions serialized on GpSimd.
- Attention softmax without max-subtraction (exp args are O(5); tol is 2e-2).
- Phase C tail: PNA aggregates per tile, PE-transposed to pnaT (bf16), then
  hxT = Wpost^T @ pnaT batched over <=4 tiles per matmul pair; global pool =
  DVE column-reduce of hxT; macro pool via host-staged multiplicity rows on
  the few blocks containing macro nodes (macro nodes sort first in their
  degree class). AllReduce + tiny MLP head as baseline.
- u (node attention logit) from a host-staged node-tile h table via DVE.
"""

import numpy as np

import bass_rust
import concourse.bass as bass
import concourse.tile as tile
from concourse import bass_isa, mybir
from concourse.bass_utils import run_bass_kernel_spmd
from concourse.library_config import mlp as MLP_LIB
from concourse.masks import make_identity
from concourse.vector_clock import ScopedClock

try:
    import ml_dtypes

    BF16_NP = np.dtype(ml_dtypes.bfloat16)
except Exception:  # pragma: no cover
    BF16_NP = None

# ----------------------------------------------------------------- constants
N_NODES = 200000
N_HE = 100000
NNZ = 1000000
F_H = 32  # 29 + 2 + 1
F_HPF = 36  # h + pin_feature
C = 64
NCORES = 8
P = 128
K_CH = 128  # slots per partition per chunk
MAX_RUN = 8
W_EF = 66  # ef table row: 64 e_feat + v + pad (bf16, 132B)
SLOPE = 0.1
N_MACRO = 512
F32 = mybir.dt.float32
BF16 = mybir.dt.bfloat16
I32 = mybir.dt.int32
I16 = mybir.dt.int16
AX = mybir.AxisListType.X
AF = mybir.ActivationFunctionType


# ------------------------------------------------------- walrus workarounds
def _patched_drain_and_barrier(self, tick_clock, wait_clock):
    nc = self.nc
    assert self.sems is not None
    handles = list(self.sems.allocated().values())
    scratch = nc.sync.sem_inc(handles[0], 0) if handles else nc.sync.drain()
    wait_clock.add_sem_waits(scratch.ins, ScopedClock({None: tick_clock.global_clock}))
    waits = list(scratch.ins.sync_info.on_wait)
    scratch.ins.sync_info = bass_rust.SyncInfo(on_wait=[], on_update=[])
    by_name = {h.name: h for h in handles}
    for w in waits:
        nc.sync.wait_ge(by_name[w.ant_name], w.wait_value)
    nc.sync.drain()
    nc.all_engine_barrier()
    popped = nc._tile_sem_poison_stack.pop()
    assert popped is self._sem_poison
    nc.clear_and_free_semaphores(handles)
    nc.all_engine_barrier()


tile.TileContext._drain_and_barrier = _patched_drain_and_barrier

_WS_CTR = [0]


def _split_waits(nc):
    """This walrus build allows at most one sync-wait per instruction; hoist
    extras onto NoOps inserted just before, same engine."""
    for fn in nc.m.functions:
        for bb in fn.blocks:
            insts = list(bb.instructions)
            new = []
            for inst in insts:
                si = inst.sync_info
                if si is not None and len(si.on_wait) > 1:
                    waits = list(si.on_wait)
                    for w in waits[:-1]:
                        _WS_CTR[0] += 1
                        new.append(
                            mybir.InstNoOp(
                                name=f"waitsplit_{_WS_CTR[0]}",
                                engine=inst.engine,
                                sync_info=mybir.SyncInfo(on_wait=[w], on_update=[]),
                                bass_nofuse=True,
                            )
                        )
                    inst.sync_info = mybir.SyncInfo(
                        on_wait=[waits[-1]], on_update=list(si.on_update)
                    )
                new.append(inst)
            bb.instructions = new


RANGE = 32768
HOP_MAX = 1024


def _load_lib_isa(nc, lib):
    sn = "NEURON_ISA_TPB_PSEUDO_LIBRARY_RELOAD_INDEX_STRUCT"
    po = nc.isa.get_enum("NEURON_ISA_TPB_PSEUDO_OPCODE")
    ant = {
        "pseudo_opcode": po.NEURON_ISA_TPB_PSEUDO_OPCODE_PSEUDO_LIBRARY_RELOAD_INDEX.value,
        "lib_index": lib.index,
    }
    b, fix = bass_isa.isa_struct(nc.isa, 223, ant, struct_name=sn)
    assert not fix
    return nc.gpsimd.add_instruction(
        mybir.InstISA(
            name=f"I-{nc.next_id()}", isa_opcode=223, engine=mybir.EngineType.Pool,
            instr=b, op_name="PseudoReloadLibraryIndex", ant_dict=ant, ins=[], outs=[],
        )
    )


def _raw_dma_gather(g, out_ap, in_ap, idxs_ap, num_idxs, elem_size, elem_step, reg):
    stride_bytes = elem_step * mybir.dt.size(in_ap.dtype)
    assert stride_bytes % 256 == 0
    g.reg_mov(reg, num_idxs)
    _in_ap = g.lower_ap_dma(in_ap, for_custom_bir_dma=True)
    _idxs_ap = g.lower_ap(idxs_ap)
    _out_ap = g.lower_ap(out_ap)
    return g.add_instruction(
        mybir.InstDMAGatherAnt(
            name=g.bass.get_next_instruction_name(),
            ins=[*_in_ap, _idxs_ap, g.lower_val_access(reg)],
            outs=[_out_ap],
            transpose=False, num_idxs=num_idxs, elem_size=elem_size,
            stride_bytes_256=stride_bytes // 256, gen_mode=0, single_packet=True,
            queue_num=0, sbuf_tokens_per_rank=0, sbuf_free_dim_per_rank=0,
            sbuf_free_dim_pad_per_rank=0, sbuf_byte_offset=0,
        )
    )


def _wrap16(vals, n):
    a = np.zeros((16, n // 16), np.int16)
    g = np.arange(len(vals))
    a[g % 16, g // 16] = vals
    return np.tile(a, (8, 1))


def _to_bf16(a):
    a = np.ascontiguousarray(a, dtype=np.float32)
    if BF16_NP is not None:
        return a.astype(BF16_NP)
    u = a.view(np.uint32)
    return ((u + 0x8000) >> 16).astype(np.uint16)


def _bf16_round(a):
    """fp32 -> nearest bf16 -> fp32 (for the numpy emulator)."""
    a = np.ascontiguousarray(a, dtype=np.float32)
    if BF16_NP is not None:
        return a.astype(BF16_NP).astype(np.float32)
    u = a.view(np.uint32)
    r = ((u + 0x8000) & 0xFFFF0000).astype(np.uint32)
    return r.view(np.float32)


# ----------------------------------------------------------- preprocessing
def _partition_by_degree(deg, ncores, first_flag=None):
    """Deal ids with deg>=1 round-robin per degree class across cores.
    first_flag: optional bool array - ids with True sort first in their class.
    """
    n = len(deg)
    if first_flag is None:
        order = np.lexsort((np.arange(n), deg))
    else:
        order = np.lexsort((np.arange(n), (~first_flag).astype(np.int8), deg))
    order = order[deg[order] >= 1]
    d_sorted = deg[order].astype(np.int64)
    change = np.nonzero(np.diff(d_sorted))[0] + 1
    starts = np.r_[0, change]
    ends = np.r_[change, len(order)]
    rank = np.arange(len(order)) - np.repeat(starts, ends - starts)
    core_of = (rank % ncores).astype(np.int32)
    lrank = rank // ncores
    tiles = []
    local = np.zeros(len(order), np.int64)
    base = 0
    for s, e in zip(starts, ends):
        d = int(d_sorted[s])
        m = int(np.ceil((e - s) / ncores))
        t_d = int(np.ceil(m / P))
        idx = slice(s, e)
        local[idx] = base + lrank[idx]
        for t in range(t_d):
            tiles.append((d, base + t * P))
        base += t_d * P
    n_rows = base + P  # final all-dummy tile (sentinel rows)
    core = np.full(n, -1, np.int32)
    loc = np.full(n, -1, np.int64)
    core[order] = core_of
    loc[order] = local
    return core, loc, tiles, n_rows


def _pack_chunks(tiles):
    place = []
    chunk, cur = 0, 0
    used = {}
    for d, _ in tiles:
        if cur + d > K_CH:
            chunk += 1
            cur = 0
        place.append((chunk, cur))
        cur += d
        used[chunk] = cur
    n_chunks = chunk + 1
    runs = []
    i = 0
    while i < len(tiles):
        d = tiles[i][0]
        ch, col = place[i]
        j = i
        while (
            j + 1 < len(tiles)
            and tiles[j + 1][0] == d
            and place[j + 1][0] == ch
            and j + 1 - i + 1 <= MAX_RUN
        ):
            j += 1
        runs.append((ch, col, d, list(range(i, j + 1))))
        i = j + 1
    return place, n_chunks, runs, used


def _rank_within(seg_ids):
    order = np.argsort(seg_ids, kind="stable")
    sorted_ids = seg_ids[order]
    change = np.nonzero(np.diff(sorted_ids))[0] + 1
    starts = np.r_[0, change]
    counts = np.diff(np.r_[starts, len(sorted_ids)])
    r = np.arange(len(sorted_ids)) - np.repeat(starts, counts)
    out = np.empty(len(seg_ids), np.int64)
    out[order] = r
    return out


def _blocks_of(runs):
    """Split runs' tiles into groups of <=4 for the hxT matmul blocks.
    Returns [(ch, d, t0, n_tiles, col_of_group)] in device order."""
    blocks = []
    for (ch, col, d, tl) in runs:
        for g0 in range(0, len(tl), 4):
            g = tl[g0 : g0 + 4]
            blocks.append((ch, d, g[0], len(g), col + g0 * d))
    return blocks


def preprocess(inputs):
    x = np.asarray(inputs["x"], np.float32)
    fake_pos = np.asarray(inputs["fake_pos"], np.float32)
    edge_index = np.asarray(inputs["edge_index"])
    pin_feature = np.asarray(inputs["pin_feature"], np.float32)
    macro_index = np.asarray(inputs["macro_index"])
    node_idx = edge_index[0].astype(np.int64)
    he_idx = edge_index[1].astype(np.int64)

    deg_n = np.bincount(node_idx, minlength=N_NODES)
    deg_e = np.bincount(he_idx, minlength=N_HE)

    ismacro = np.zeros(N_NODES, np.float32)
    ismacro[macro_index] = 1.0
    mult = np.bincount(macro_index, minlength=N_NODES).astype(np.float32)

    core_n, loc_n, tiles_n, NRN = _partition_by_degree(
        deg_n, NCORES, first_flag=mult > 0
    )
    core_e, loc_e, tiles_e, NRE = _partition_by_degree(deg_e, NCORES)
    placeA, nchA, runsA, usedA = _pack_chunks(tiles_e)
    placeC, nchC, runsC, usedC = _pack_chunks(tiles_n)
    TN = len(tiles_n)
    TE = len(tiles_e)

    erow = core_e.astype(np.int64) * NRE + loc_e  # ef_full row per hyperedge
    sent_g = TE * P  # first dummy-tile row of shard 0 (zeroed on device)

    h_full = np.concatenate([x, fake_pos, ismacro[:, None]], 1)  # (N, 32)

    # --- per-pin placement, hyperedge-major (phase A) ---
    jA = _rank_within(he_idx)
    cA = core_e[he_idx]
    tA = loc_e[he_idx] // P
    pA = loc_e[he_idx] % P
    chA = np.array([pl[0] for pl in placeA], np.int64)[tA]
    colA = np.array([pl[1] for pl in placeA], np.int64)[tA] + jA

    # --- node-major (phase C) ---
    jC = _rank_within(node_idx)
    cC = core_n[node_idx]
    tC = loc_n[node_idx] // P
    pC = loc_n[node_idx] % P
    chC = np.array([pl[0] for pl in placeC], np.int64)[tC]
    colC = np.array([pl[1] for pl in placeC], np.int64)[tC] + jC

    blocksC = _blocks_of(runsC)

    per_core = []
    for c in range(NCORES):
        # ---- phase A staging: hpf [nchA, P, K_CH, 36] bf16
        hpf = np.zeros((nchA, P, K_CH, F_HPF), np.float32)
        m = cA == c
        hpf[chA[m], pA[m], colA[m], :F_H] = h_full[node_idx[m]]
        hpf[chA[m], pA[m], colA[m], F_H:] = pin_feature[m]

        # ---- phase C staging: int32 ef_full row per slot
        m2 = cC == c
        cidx = np.full((nchC, P, K_CH), sent_g, np.int32)
        cidx[chC[m2], pC[m2], colC[m2]] = erow[he_idx[m2]].astype(np.int32)

        # ---- hn: node-tile h table [P, TN, 33] (col 32 = 1 for real rows)
        hn = np.zeros((P, TN, F_H + 1), np.float32)
        sel = core_n == c
        nl = loc_n[sel]
        hn[nl % P, nl // P, :F_H] = h_full[sel]
        hn[nl % P, nl // P, F_H] = 1.0

        # ---- macro multiplicity rows per hxT block
        multn = np.zeros((P, TN), np.float32)
        multn[nl % P, nl // P] = mult[sel]
        mrep_blocks = {}
        for bi, (ch, d, t0, G, col) in enumerate(blocksC):
            w = np.concatenate([multn[:, t0 + ti] for ti in range(G)])
            if w.any():
                mrep_blocks[bi] = w

        per_core.append(
            dict(
                hpf=hpf,
                cidx=cidx,
                hn=hn,
                mrep_blocks=mrep_blocks,
                multn=multn,
            )
        )

    n_ranges = int(np.ceil(NCORES * NRE / RANGE))
    counts = np.zeros((nchC, n_ranges, NCORES), np.int64)
    percore_sorted = []
    for c in range(NCORES):
        pc = per_core[c]
        lst = []
        for ch in range(nchC):
            used = usedC[ch]
            rows_flat = pc["cidx"][ch][:, :used].T.reshape(-1).astype(np.int64)
            order = np.argsort(rows_flat, kind="stable")
            srows = rows_flat[order]
            for r in range(n_ranges):
                counts[ch, r, c] = int(((srows >= r * RANGE) & (srows < (r + 1) * RANGE)).sum())
            lst.append((rows_flat, order, srows))
        percore_sorted.append(lst)

    gplan = []
    for ch in range(nchC):
        pads = []
        for r in range(n_ranges):
            m = int(counts[ch, r].max())
            pads.append(int(np.ceil(m / P)) * P)
        h1_parts = []
        base = 0
        for r in range(n_ranges):
            off = 0
            while off < pads[r]:
                n = min(HOP_MAX, pads[r] - off)
                h1_parts.append((r, base + off, n))
                off += n
            base += pads[r]
        total = base
        n2 = usedC[ch] * P
        h2_parts = []
        off = 0
        while off < n2:
            n = min(HOP_MAX, n2 - off)
            h2_parts.append((off, n))
            off += n
        gplan.append(dict(pads=pads, h1=h1_parts, total=total, n2=n2, h2=h2_parts))

    for c in range(NCORES):
        pc = per_core[c]
        pc["i1"] = {}
        pc["i2"] = {}
        for ch in range(nchC):
            rows_flat, order, srows = percore_sorted[c][ch]
            plan = gplan[ch]
            bounce_row = np.empty(len(rows_flat), np.int64)
            tok_vals = np.zeros(plan["total"], np.int64)
            base = 0
            ptr = 0
            for r in range(n_ranges):
                cnt = int(counts[ch, r, c])
                pad = plan["pads"][r]
                tok_vals[base : base + cnt] = srows[ptr : ptr + cnt] - r * RANGE
                bounce_row[order[ptr : ptr + cnt]] = base + np.arange(cnt)
                ptr += cnt
                base += pad
            assert ptr == len(rows_flat) and base == plan["total"]
            pc["i1"][ch] = _wrap16(tok_vals.astype(np.int16), plan["total"])
            pc["i2"][ch] = _wrap16(bounce_row.astype(np.int16), plan["n2"])

    # union of macro blocks across cores (device program is shared)
    mblk_union = {}
    off = 0
    for bi in sorted(set().union(*[set(pc["mrep_blocks"]) for pc in per_core])):
        G = blocksC[bi][3]
        mblk_union[bi] = (off, G * P)
        off += G * P
    MRC = max(off, P)
    for pc in per_core:
        mrep = np.zeros((C, MRC), np.float32)
        for bi, w in pc["mrep_blocks"].items():
            o, n = mblk_union[bi]
            mrep[:, o : o + len(w)] = w[None, :]
        pc["mrep"] = mrep
        pc["hpf_b"] = np.ascontiguousarray(
            pc["hpf"].reshape(nchA, P, K_CH * F_HPF), dtype=np.float32
        )
        pc["hn_b"] = _to_bf16(pc["hn"].reshape(P, TN * (F_H + 1)))
        pc["mrep_b"] = _to_bf16(mrep)

    return dict(
        per_core=per_core,
        tiles_n=tiles_n,
        tiles_e=tiles_e,
        runsA=runsA,
        runsC=runsC,
        usedA=usedA,
        usedC=usedC,
        nchA=nchA,
        nchC=nchC,
        NRN=NRN,
        NRE=NRE,
        TN=TN,
        TE=TE,
        sent_g=sent_g,
        blocksC=blocksC,
        mblk_union=mblk_union,
        MRC=MRC,
        core_n=core_n,
        loc_n=loc_n,
        core_e=core_e,
        loc_e=loc_e,
        erow=erow,
        n_ranges=n_ranges,
        gplan=gplan,
    )


def _const_col_factory(nc, pool):
    cache = {}

    def cc(val, parts=P):
        key = (float(val), parts)
        if key not in cache:
            t = pool.tile([parts, 1], F32, tag=f"cc{len(cache)}")
            nc.vector.memset(t[:], float(val))
            cache[key] = t
        return cache[key][:]

    return cc


# ----------------------------------------------------------- device program
def build_program(prep, debug=False):
    NRE = prep["NRE"]
    nchA, nchC = prep["nchA"], prep["nchC"]
    runsA, runsC = prep["runsA"], prep["runsC"]
    TN, TE = prep["TN"], prep["TE"]
    blocksC = prep["blocksC"]
    mblk_union = prep["mblk_union"]
    MRC = prep["MRC"]
    core_ids = list(range(NCORES))

    nc = bass.Bass("TRN2", target_bir_lowering=False, debug=False, num_devices=NCORES)

    # ---- inputs
    gplan = prep["gplan"]
    hpf_in = nc.declare_dram_parameter("hpf", [nchA, P, K_CH * F_HPF], F32, isOutput=False)
    i1_in = {
        ch: nc.declare_dram_parameter(f"i1_{ch}", [P, gplan[ch]["total"] // 16], I16, isOutput=False)
        for ch in range(nchC)
    }
    i2_in = {
        ch: nc.declare_dram_parameter(f"i2_{ch}", [P, gplan[ch]["n2"] // 16], I16, isOutput=False)
        for ch in range(nchC)
    }
    hn_in = nc.declare_dram_parameter("hn", [P, TN * (F_H + 1)], BF16, isOutput=False)
    mrep_in = nc.declare_dram_parameter("mrep", [C, MRC], BF16, isOutput=False)
    WR_in = nc.declare_dram_parameter("WR", [F_HPF + 1, C], F32, isOutput=False)
    WRT_in = nc.declare_dram_parameter("WRT", [C, F_HPF + 1], F32, isOutput=False)
    att2_in = nc.declare_dram_parameter("att2", [C, 1], F32, isOutput=False)
    att1_in = nc.declare_dram_parameter("att1", [C, 1], F32, isOutput=False)
    W1bT_in = nc.declare_dram_parameter("W1bT", [C, F_H + 1], F32, isOutput=False)
    Wpost_in = nc.declare_dram_parameter("Wpost", [4 * C, C], F32, isOutput=False)
    bpostT_in = nc.declare_dram_parameter("bpostT", [C, 1], F32, isOutput=False)
    Wm1_in = nc.declare_dram_parameter("Wm1", [2 * C, C], F32, isOutput=False)
    bm1_in = nc.declare_dram_parameter("bm1", [1, C], F32, isOutput=False)
    Wm2_in = nc.declare_dram_parameter("Wm2", [C, C // 2], F32, isOutput=False)
    bm2_in = nc.declare_dram_parameter("bm2", [1, C // 2], F32, isOutput=False)
    Wm3_in = nc.declare_dram_parameter("Wm3", [C // 2, 1], F32, isOutput=False)
    bm3_in = nc.declare_dram_parameter("bm3", [1, 1], F32, isOutput=False)
    z_out = nc.declare_dram_parameter("z", [1, 1], F32, isOutput=True)
    if debug:
        ef_dbg = nc.declare_dram_parameter("ef_dbg", [NRE, W_EF], BF16, isOutput=True)
        u_dbg = nc.declare_dram_parameter("u_dbg", [P, TN], F32, isOutput=True)
        pool_dbg = nc.declare_dram_parameter("pool_dbg", [2, C], F32, isOutput=True)

    # ---- internal DRAM
    R0 = (NCORES * NRE + P - 1) // P
    PADR = P * R0
    ef_shard = nc.dram_tensor("ef_shard", [NRE, W_EF], BF16)
    ef_full = nc.dram_tensor("ef_full", [PADR, W_EF], BF16, addr_space="Shared")
    ef256 = nc.dram_tensor("ef256", [PADR, 128], BF16)
    ar_in = nc.dram_tensor("ar_in", [2, C], F32)
    ar_out = nc.dram_tensor("ar_out", [2, C], F32, addr_space="Shared")

    # persistent SBUF (used across contexts)
    u_sb = nc.alloc_sbuf_tensor("u_sb", [P, TN], F32)

    with nc.Block() as blk_lib:

        @blk_lib.gpsimd
        def _(g):
            _load_lib_isa(nc, MLP_LIB)

    # =========== context 1: consts + u + phase A (e_feat) ===========
    with tile.TileContext(nc) as tc:
        with (
            tc.tile_pool(name="acons", bufs=1) as acons,
            tc.tile_pool(name="ahpf", bufs=3) as ahpf,
            tc.tile_pool(name="arun", bufs=3) as arun,
            tc.tile_pool(name="apsum", bufs=1, space="PSUM") as apsum,
            tc.tile_pool(name="apsum2", bufs=2, space="PSUM") as apsum2,
        ):
            ident = acons.tile([P, P], F32)
            make_identity(nc, ident[:])
            ccA = _const_col_factory(nc, acons)

            # R = [[W1],[Wpin],[b1]] with 65th column = R @ att2
            R = acons.tile([F_HPF + 1, C + 1], F32)
            nc.sync.dma_start(out=R[:, :C], in_=WR_in[:, :])
            wrt = acons.tile([C, F_HPF + 1], F32)
            nc.sync.dma_start(out=wrt[:], in_=WRT_in[:, :])
            att2 = acons.tile([C, 1], F32)
            nc.sync.dma_start(out=att2[:], in_=att2_in[:, :])
            cps = apsum.tile([P, P], F32, space="PSUM", tag="cps")
            nc.tensor.matmul(
                cps[: F_HPF + 1, :1], lhsT=wrt[:], rhs=att2[:], start=True, stop=True
            )
            nc.scalar.activation(R[:, C : C + 1], cps[: F_HPF + 1, :1], AF.Copy)

            # w_u broadcast to all partitions: wur[p, f] = (W1b @ att1)[f]
            w1bt = acons.tile([C, F_H + 1], F32)
            nc.sync.dma_start(out=w1bt[:], in_=W1bT_in[:, :])
            att1 = acons.tile([C, 1], F32)
            nc.sync.dma_start(out=att1[:], in_=att1_in[:, :])
            cps2 = apsum.tile([P, P], F32, space="PSUM", tag="cps")
            nc.tensor.matmul(
                cps2[: F_H + 1, :1], lhsT=w1bt[:], rhs=att1[:], start=True, stop=True
            )
            wu = acons.tile([F_H + 1, 1], F32)
            nc.vector.tensor_copy(wu[:], cps2[: F_H + 1, :1])
            cps3 = apsum.tile([P, P], F32, space="PSUM", tag="cps")
            nc.tensor.transpose(
                out=cps3[:1, : F_H + 1], in_=wu[:], identity=ident[: F_H + 1, : F_H + 1]
            )
            wuT = acons.tile([1, F_H + 1], F32)
            nc.vector.tensor_copy(wuT[:], cps3[:1, : F_H + 1])
            ones1 = acons.tile([1, P], F32)
            nc.vector.memset(ones1[:], 1.0)
            cps4 = apsum.tile([P, P], F32, space="PSUM", tag="cps")
            nc.tensor.matmul(
                cps4[:, : F_H + 1], lhsT=ones1[:], rhs=wuT[:], start=True, stop=True
            )
            wur = acons.tile([P, F_H + 1], BF16)
            nc.scalar.activation(wur[:], cps4[:, : F_H + 1], AF.Copy)

            # u = reduce(hn * w_u_rep) over f, in tile-blocks
            hn_sb = acons.tile([P, TN * (F_H + 1)], BF16)
            nc.sync.dma_start(out=hn_sb[:], in_=hn_in[:, :])
            UB = 32
            for b0 in range(0, TN, UB):
                nb = min(UB, TN - b0)
                tmp = arun.tile([P, UB * (F_H + 1)], F32, tag="utmp")
                nc.vector.tensor_tensor(
                    out=tmp[:, : nb * (F_H + 1)].rearrange("p (t f) -> p t f", t=nb),
                    in0=hn_sb[:, b0 * (F_H + 1) : (b0 + nb) * (F_H + 1)].rearrange(
                        "p (t f) -> p t f", t=nb
                    ),
                    in1=wur[:, None, :].to_broadcast([P, nb, F_H + 1]),
                    op=mybir.AluOpType.mult,
                )
                nc.vector.reduce_sum(
                    u_sb[:, b0 : b0 + nb],
                    tmp[:, : nb * (F_H + 1)].rearrange("p (t f) -> p t f", t=nb),
                    axis=AX,
                )

            # zero the sentinel (dummy) tile rows of ef_shard
            zt = acons.tile([P, W_EF], BF16)
            nc.vector.memset(zt[:], 0.0)
            nc.sync.dma_start(out=ef_shard[TE * P : (TE + 1) * P, :], in_=zt[:])

            # ---- phase A main loop
            run_by_chunkA = {}
            for r in runsA:
                run_by_chunkA.setdefault(r[0], []).append(r)

            for ch in range(nchA):
                hpf_t = ahpf.tile([P, K_CH * F_HPF], F32)
                nc.sync.dma_start(out=hpf_t[:], in_=hpf_in[ch])
                for (_, col, d, tl) in run_by_chunkA.get(ch, []):
                    T = len(tl)
                    t0 = tl[0]
                    FW = F_HPF + 1
                    hs = arun.tile([P, MAX_RUN * FW], F32, tag="hs")
                    hs3 = hs[:, : T * FW].rearrange("p (t f) -> p t f", t=T)
                    nc.vector.reduce_sum(
                        hs3[:, :, :F_HPF],
                        hpf_t[:, col * F_HPF : (col + T * d) * F_HPF].rearrange(
                            "p (t j f) -> p t f j", t=T, j=d
                        ),
                        axis=AX,
                    )
                    nc.vector.memset(hs3[:, :, F_HPF], float(d))
                    nc.scalar.activation(
                        hs[:, : T * FW], hs[:, : T * FW], AF.Copy, scale=ccA(1.0 / d)
                    )
                    efb = arun.tile([P, MAX_RUN * (C + 1)], BF16, tag="efb")
                    for ti in range(T):
                        hsT_ps = apsum2.tile([P, P], F32, space="PSUM", tag="hsT")
                        nc.tensor.transpose(
                            out=hsT_ps[:FW, :],
                            in_=hs[:, ti * FW : (ti + 1) * FW],
                            identity=ident[:],
                        )
                        hsT = arun.tile([FW, P], F32, tag="hsTs")
                        nc.scalar.activation(hsT[:], hsT_ps[:FW, :], AF.Copy)
                        ef_ps = apsum2.tile([P, C + 1], F32, space="PSUM", tag="efps")
                        nc.tensor.matmul(
                            ef_ps[:], lhsT=hsT[:], rhs=R[:], start=True, stop=True
                        )
                        nc.scalar.activation(
                            efb[:, ti * (C + 1) : (ti + 1) * (C + 1)], ef_ps[:], AF.Copy
                        )
                    nc.sync.dma_start(
                        out=ef_shard[t0 * P : (t0 + T) * P, : C + 1].rearrange(
                            "(t p) w -> p t w", p=P
                        ),
                        in_=efb[:, : T * (C + 1)].rearrange("p (t w) -> p t w", t=T),
                    )

    # =========== AllGather ef ===========
    with (
        nc.semaphore("ag_sem") as ag_sem,
        nc.Block() as blk,
    ):

        @blk.gpsimd
        def _(g):
            g.collective_compute(
                "AllGather",
                mybir.AluOpType.bypass,
                replica_groups=[core_ids],
                ins=[ef_shard[:, :]],
                outs=[ef_full[: NCORES * NRE, :]],
            ).then_inc(ag_sem, 1)
            g.wait_ge(ag_sem, 1)

    # =========== expand ef_full -> 256B-stride ef256 ===========
    NEXP = 8
    RPE = (R0 + NEXP - 1) // NEXP
    with tile.TileContext(nc) as tc:
        with tc.tile_pool(name="expp", bufs=2) as expp:
            for k in range(NEXP):
                ncols = min(RPE, R0 - k * RPE)
                if ncols <= 0:
                    break
                t = expp.tile([P, RPE * W_EF], BF16, tag="exp")
                nc.sync.dma_start(
                    out=t[:, : ncols * W_EF].rearrange("p (i w) -> p i w", w=W_EF),
                    in_=ef_full[:, :].rearrange("(p i) w -> p i w", p=P)[:, k * RPE : k * RPE + ncols],
                )
                nc.sync.dma_start(
                    out=ef256[:, :W_EF].rearrange("(p i) w -> p i w", p=P)[:, k * RPE : k * RPE + ncols],
                    in_=t[:, : ncols * W_EF].rearrange("p (i w) -> p i w", w=W_EF),
                )

    # =========== context 2: phase C ===========
    NBLK = len(blocksC)
    with tile.TileContext(nc) as tc:
        with (
            tc.tile_pool(name="ccons", bufs=1) as ccons,
            tc.tile_pool(name="cidxp", bufs=4) as cidxp,
            tc.tile_pool(name="ch1", bufs=1) as ch1p,
            tc.tile_pool(name="cbnc", bufs=2, space="DRAM") as cbnc,
            tc.tile_pool(name="cstg", bufs=2) as cstg,
            tc.tile_pool(name="cmsg", bufs=1) as cmsg,
            tc.tile_pool(name="csml", bufs=2) as csml,
            tc.tile_pool(name="cpost", bufs=2) as cpost,
            tc.tile_pool(name="cpT", bufs=2) as cpT,
            tc.tile_pool(name="chxp", bufs=2) as chxp,
            tc.tile_pool(name="ctpsum", bufs=4, space="PSUM") as ctpsum,
            tc.tile_pool(name="chpsum", bufs=1, space="PSUM") as chpsum,
        ):
            identC = ccons.tile([P, P], F32)
            make_identity(nc, identC[:])
            ccC = _const_col_factory(nc, ccons)
            wpost_a = ccons.tile([P, C], F32)
            nc.sync.dma_start(out=wpost_a[:], in_=Wpost_in[:P, :])
            wpost_b = ccons.tile([P, C], F32)
            nc.sync.dma_start(out=wpost_b[:], in_=Wpost_in[P:, :])
            bpostT = ccons.tile([C, 1], F32)
            nc.sync.dma_start(out=bpostT[:], in_=bpostT_in[:, :])
            mrep_sb = ccons.tile([C, MRC], BF16)
            nc.sync.dma_start(out=mrep_sb[:], in_=mrep_in[:, :])
            poolg = ccons.tile([C, NBLK + 1], F32)
            poolm = ccons.tile([C, NBLK + 1], F32)
            nc.vector.memset(poolg[:], 0.0)
            nc.vector.memset(poolm[:], 0.0)

            run_by_chunkC = {}
            for r in runsC:
                run_by_chunkC.setdefault(r[0], []).append(r)

            bi_ctr = [0]

            nreg = nc.gpsimd.alloc_register()
            for ch in range(nchC):
                plan = gplan[ch]
                H1C = plan["total"] // P
                h1 = ch1p.tile([P, H1C * W_EF], BF16, tag="h1")
                for (r, o, n) in plan["h1"]:
                    it1 = cidxp.tile([P, HOP_MAX // 16], I16, tag="i1")
                    nc.sync.dma_start(
                        out=it1[:, : n // 16], in_=i1_in[ch][:, o // 16 : (o + n) // 16]
                    )
                    _raw_dma_gather(
                        nc.gpsimd,
                        h1[:, (o // P) * W_EF : ((o + n) // P) * W_EF].rearrange(
                            "p (i e) -> p i e", e=W_EF
                        ),
                        ef256[r * RANGE :, :W_EF],
                        it1[:, : n // 16],
                        n, W_EF, 128, nreg,
                    )
                bounce = cbnc.tile([plan["total"], 128], BF16, tag="bnc")
                nc.sync.dma_start(
                    out=bounce[:, :W_EF].rearrange("(i p) w -> p i w", p=P),
                    in_=h1[:].rearrange("p (i w) -> p i w", w=W_EF),
                )
                stg = cstg.tile([P, K_CH * W_EF], BF16)
                for (o, n) in plan["h2"]:
                    it2 = cidxp.tile([P, HOP_MAX // 16], I16, tag="i2")
                    nc.sync.dma_start(
                        out=it2[:, : n // 16], in_=i2_in[ch][:, o // 16 : (o + n) // 16]
                    )
                    _raw_dma_gather(
                        nc.gpsimd,
                        stg[:, (o // P) * W_EF : ((o + n) // P) * W_EF].rearrange(
                            "p (i e) -> p i e", e=W_EF
                        ),
                        bounce[:, :W_EF],
                        it2[:, : n // 16],
                        n, W_EF, 128, nreg,
                    )

                for (_, col, d, tl) in run_by_chunkC.get(ch, []):
                    T = len(tl)
                    F = T * d
                    t0 = tl[0]
                    stg3 = stg[:, col * W_EF : (col + F) * W_EF].rearrange(
                        "p (s w) -> p s w", s=F
                    )
                    ef_ap = stg3[:, :, :C]
                    v_sc = stg3[:, :, C]

                    # vv = fp32 copy of v column; a = lrelu(u + v); ex = exp(a)
                    vv = csml.tile([P, K_CH], F32, tag="vv")
                    nc.vector.tensor_copy(vv[:, :F], v_sc)
                    asl = csml.tile([P, K_CH], F32, tag="asl")
                    nc.vector.tensor_tensor(
                        out=asl[:, :F].rearrange("p (t j) -> p t j", t=T),
                        in0=u_sb[:, t0 : t0 + T, None].to_broadcast([P, T, d]),
                        in1=vv[:, :F].rearrange("p (t j) -> p t j", t=T),
                        op=mybir.AluOpType.add,
                    )
                    a2 = csml.tile([P, K_CH], F32, tag="a2")
                    nc.vector.tensor_scalar_mul(a2[:, :F], asl[:, :F], SLOPE)
                    nc.vector.tensor_tensor(
                        out=asl[:, :F], in0=asl[:, :F], in1=a2[:, :F],
                        op=mybir.AluOpType.max,
                    )
                    ex = csml.tile([P, K_CH], F32, tag="ex")
                    nc.scalar.activation(ex[:, :F], asl[:, :F], AF.Exp)
                    den = csml.tile([P, MAX_RUN], F32, tag="den")
                    nc.vector.reduce_sum(
                        den[:, :T],
                        ex[:, :F].rearrange("p (t j) -> p t j", t=T),
                        axis=AX,
                    )
                    nc.vector.reciprocal(den[:, :T], den[:, :T])
                    alpha = csml.tile([P, K_CH], BF16, tag="alpha")
                    nc.vector.tensor_tensor(
                        out=alpha[:, :F].rearrange("p (t j) -> p t j", t=T),
                        in0=ex[:, :F].rearrange("p (t j) -> p t j", t=T),
                        in1=den[:, :T, None].to_broadcast([P, T, d]),
                        op=mybir.AluOpType.mult,
                    )
                    # msg = alpha * e_g (bf16)
                    msg = cmsg.tile([P, K_CH * C], F32, tag="msg")
                    nc.vector.tensor_tensor(
                        out=msg[:, : F * C].rearrange("p (s e) -> p s e", s=F),
                        in0=ef_ap,
                        in1=alpha[:, :F, None].to_broadcast([P, F, C]),
                        op=mybir.AluOpType.mult,
                    )
                    msg3 = msg[:, : F * C].rearrange(
                        "p (t j e) -> p t e j", t=T, j=d, e=C
                    )
                    post = cpost.tile([P, MAX_RUN * 4 * C], F32, tag="post")
                    post3 = post[:, : T * 4 * C].rearrange(
                        "p (t q e) -> p t q e", t=T, q=4
                    )
                    nc.vector.reduce_max(post3[:, :, 1], msg3, axis=AX)
                    nc.vector.tensor_reduce(
                        post3[:, :, 2], msg3, op=mybir.AluOpType.min, axis=AX
                    )
                    sm = cpost.tile([P, MAX_RUN * C], F32, tag="sm")
                    nc.vector.reduce_sum(
                        sm[:, : T * C].rearrange("p (t e) -> p t e", t=T), msg3, axis=AX
                    )
                    nc.scalar.activation(
                        post3[:, :, 0],
                        sm[:, : T * C].rearrange("p (t e) -> p t e", t=T),
                        AF.Copy,
                        scale=ccC(1.0 / d),
                    )
                    # msq = msg^2, then sumsq
                    msq = cmsg.tile([P, K_CH * C], F32, tag="msq")
                    nc.scalar.activation(msq[:, : F * C], msg[:, : F * C], AF.Square)
                    msq3 = msq[:, : F * C].rearrange(
                        "p (t j e) -> p t e j", t=T, j=d, e=C
                    )
                    sq = cpost.tile([P, MAX_RUN * C], F32, tag="sq")
                    nc.vector.reduce_sum(
                        sq[:, : T * C].rearrange("p (t e) -> p t e", t=T), msq3, axis=AX
                    )
                    m2 = cpost.tile([P, MAX_RUN * C], F32, tag="m2")
                    nc.vector.tensor_tensor(
                        out=m2[:, : T * C].rearrange("p (t e) -> p t e", t=T),
                        in0=post3[:, :, 0],
                        in1=post3[:, :, 0],
                        op=mybir.AluOpType.mult,
                    )
                    nc.vector.tensor_scalar(
                        out=sq[:, : T * C], in0=sq[:, : T * C],
                        scalar1=1.0 / d, scalar2=None, op0=mybir.AluOpType.mult,
                    )
                    nc.vector.tensor_tensor(
                        out=sq[:, : T * C], in0=sq[:, : T * C], in1=m2[:, : T * C],
                        op=mybir.AluOpType.subtract,
                    )
                    nc.vector.tensor_scalar_max(sq[:, : T * C], sq[:, : T * C], 0.0)
                    nc.scalar.activation(
                        post3[:, :, 3],
                        sq[:, : T * C].rearrange("p (t e) -> p t e", t=T),
                        AF.Sqrt,
                        bias=ccC(1e-12),
                    )

                    # hxT blocks of <=4 tiles
                    for g0 in range(0, T, 4):
                        G = min(4, T - g0)
                        bi = bi_ctr[0]
                        bi_ctr[0] += 1
                        assert blocksC[bi][2] == tl[g0], (bi, blocksC[bi], tl, g0)
                        pT = cpT.tile([P, 2 * 4 * P], F32, tag="pT")
                        for ti in range(G):
                            ta = g0 + ti
                            for half in range(2):
                                tp_ps = ctpsum.tile([P, P], F32, space="PSUM", tag="tp")
                                nc.tensor.transpose(
                                    out=tp_ps[:],
                                    in_=post[
                                        :,
                                        ta * 4 * C + half * P : ta * 4 * C
                                        + half * P
                                        + P,
                                    ],
                                    identity=identC[:],
                                )
                                nc.scalar.activation(
                                    pT[
                                        :,
                                        half * 4 * P + ti * P : half * 4 * P
                                        + (ti + 1) * P,
                                    ],
                                    tp_ps[:],
                                    AF.Copy,
                                )
                        hx_ps = chpsum.tile([C, 4 * P], F32, space="PSUM", tag="hx")
                        nc.tensor.matmul(
                            hx_ps[:, : G * P],
                            lhsT=wpost_a[:],
                            rhs=pT[:, : G * P],
                            start=True,
                            stop=False,
                        )
                        nc.tensor.matmul(
                            hx_ps[:, : G * P],
                            lhsT=wpost_b[:],
                            rhs=pT[:, 4 * P : 4 * P + G * P],
                            start=False,
                            stop=True,
                        )
                        hx = chxp.tile([C, 4 * P], F32, tag="hx")
                        nc.vector.tensor_tensor(
                            out=hx[:, : G * P],
                            in0=hx_ps[:, : G * P],
                            in1=bpostT[:].to_broadcast([C, G * P]),
                            op=mybir.AluOpType.add,
                        )
                        hx2 = chxp.tile([C, 4 * P], F32, tag="hx2")
                        nc.vector.tensor_scalar_mul(
                            hx2[:, : G * P], hx[:, : G * P], SLOPE
                        )
                        nc.vector.tensor_tensor(
                            out=hx[:, : G * P], in0=hx[:, : G * P],
                            in1=hx2[:, : G * P], op=mybir.AluOpType.max,
                        )
                        nc.vector.reduce_sum(
                            poolg[:, bi : bi + 1], hx[:, None, : G * P], axis=AX
                        )
                        if bi in mblk_union:
                            o, ncols = mblk_union[bi]
                            assert ncols == G * P
                            hxm = chxp.tile([C, 4 * P], F32, tag="hxm")
                            nc.vector.tensor_tensor(
                                out=hxm[:, : G * P],
                                in0=hx[:, : G * P],
                                in1=mrep_sb[:, o : o + G * P],
                                op=mybir.AluOpType.mult,
                            )
                            nc.vector.reduce_sum(
                                poolm[:, bi : bi + 1], hxm[:, None, : G * P], axis=AX
                            )

            assert bi_ctr[0] == NBLK
            # final pool reduction -> ar_in [2, C]
            pg = ccons.tile([C, 1], F32)
            nc.vector.reduce_sum(pg[:], poolg[:, None, :], axis=AX)
            pm = ccons.tile([C, 1], F32)
            nc.vector.reduce_sum(pm[:], poolm[:, None, :], axis=AX)
            pools2 = ccons.tile([C, 2], F32)
            nc.vector.tensor_copy(pools2[:, 0:1], pm[:])
            nc.vector.tensor_copy(pools2[:, 1:2], pg[:])
            poolsT_ps = chpsum.tile([2, C], F32, space="PSUM", tag="pt")
            nc.tensor.transpose(out=poolsT_ps[:], in_=pools2[:], identity=identC[:C, :C])
            poolsT = ccons.tile([2, C], F32)
            nc.vector.tensor_copy(poolsT[:], poolsT_ps[:])
            nc.sync.dma_start(out=ar_in[:, :], in_=poolsT[:])

    if debug:
        with (
            nc.semaphore("dbg_sem") as dbg_sem,
            nc.Block() as blkd,
        ):

            @blkd.gpsimd
            def _(g):
                g.dma_start(out=ef_dbg[:, :], in_=ef_shard[:, :]).then_inc(dbg_sem, 16)
                g.dma_start(out=pool_dbg[:, :], in_=ar_in[:, :]).then_inc(dbg_sem, 16)
                g.wait_ge(dbg_sem, 32)

        with tile.TileContext(nc) as tc:
            with tc.tile_pool(name="dbgp", bufs=1) as dbgp:
                ut = dbgp.tile([P, TN], F32)
                nc.vector.tensor_copy(ut[:], u_sb[:])
                nc.sync.dma_start(out=u_dbg[:, :], in_=ut[:])

    # =========== AllReduce pools ===========
    with (
        nc.semaphore("ar_sem") as ar_sem,
        nc.Block() as blk3,
    ):

        @blk3.gpsimd
        def _(g):
            g.collective_compute(
                "AllReduce",
                mybir.AluOpType.add,
                replica_groups=[core_ids],
                ins=[ar_in[:, :]],
                outs=[ar_out[:, :]],
            ).then_inc(ar_sem, 1)
            g.wait_ge(ar_sem, 1)

    # =========== context 3: MLP head ===========
    with tile.TileContext(nc) as tc:
        with (
            tc.tile_pool(name="mpool", bufs=1) as mpool,
            tc.tile_pool(name="mpsum", bufs=1, space="PSUM") as mpsum,
        ):
            identM = mpool.tile([P, P], F32)
            make_identity(nc, identM[:])
            onesM = mpool.tile([1, 1], F32)
            nc.vector.memset(onesM[:], 1.0)
            ccM = _const_col_factory(nc, mpool)
            pool2 = mpool.tile([2, C], F32)
            nc.sync.dma_start(out=pool2[:], in_=ar_out[:, :])
            poolT_ps = mpsum.tile([P, P], F32, space="PSUM")
            nc.tensor.transpose(out=poolT_ps[:C, :2], in_=pool2[:], identity=identM[:2, :2])
            pooled = mpool.tile([P, 1], F32)
            nc.scalar.activation(
                pooled[:C, :], poolT_ps[:C, :1], AF.Copy, scale=ccM(1.0 / N_MACRO, C)
            )
            nc.scalar.activation(
                pooled[C:, :], poolT_ps[:C, 1:2], AF.Copy, scale=ccM(1.0 / N_NODES, C)
            )
            wm1 = mpool.tile([2 * C, C], F32)
            nc.sync.dma_start(out=wm1[:], in_=Wm1_in[:, :])
            bm1 = mpool.tile([1, C], F32)
            nc.sync.dma_start(out=bm1[:], in_=bm1_in[:, :])
            wm2 = mpool.tile([C, C // 2], F32)
            nc.sync.dma_start(out=wm2[:], in_=Wm2_in[:, :])
            bm2 = mpool.tile([1, C // 2], F32)
            nc.sync.dma_start(out=bm2[:], in_=bm2_in[:, :])
            wm3 = mpool.tile([C // 2, 1], F32)
            nc.sync.dma_start(out=wm3[:], in_=Wm3_in[:, :])
            bm3 = mpool.tile([1, 1], F32)
            nc.sync.dma_start(out=bm3[:], in_=bm3_in[:, :])

            def _lrelu_row(dst, src_ps, width):
                tmp = mpool.tile([1, width], F32, tag=f"lr{width}")
                nc.scalar.activation(tmp[:], src_ps[:], AF.Copy, scale=ccM(SLOPE, 1))
                nc.vector.tensor_tensor(
                    out=dst[:], in0=src_ps[:], in1=tmp[:], op=mybir.AluOpType.max
                )

            z1_ps = mpsum.tile([1, C], F32, space="PSUM")
            nc.tensor.matmul(z1_ps[:], lhsT=pooled[:], rhs=wm1[:], start=True, stop=False)
            nc.tensor.matmul(
                z1_ps[:], lhsT=onesM[:].to_broadcast([1, 1]), rhs=bm1[:],
                start=False, stop=True,
            )
            z1 = mpool.tile([1, C], F32)
            _lrelu_row(z1, z1_ps, C)
            z1T_ps = mpsum.tile([P, P], F32, space="PSUM")
            nc.tensor.transpose(out=z1T_ps[:C, :1], in_=z1[:], identity=identM[:1, :1])
            z1T = mpool.tile([C, 1], F32)
            nc.vector.tensor_copy(z1T[:], z1T_ps[:C, :1])
            z2_ps = mpsum.tile([1, C // 2], F32, space="PSUM")
            nc.tensor.matmul(z2_ps[:], lhsT=z1T[:], rhs=wm2[:], start=True, stop=False)
            nc.tensor.matmul(
                z2_ps[:], lhsT=onesM[:].to_broadcast([1, 1]), rhs=bm2[:],
                start=False, stop=True,
            )
            z2 = mpool.tile([1, C // 2], F32)
            _lrelu_row(z2, z2_ps, C // 2)
            z2T_ps = mpsum.tile([P, P], F32, space="PSUM")
            nc.tensor.transpose(out=z2T_ps[: C // 2, :1], in_=z2[:], identity=identM[:1, :1])
            z2T = mpool.tile([C // 2, 1], F32)
            nc.vector.tensor_copy(z2T[:], z2T_ps[: C // 2, :1])
            z3_ps = mpsum.tile([1, 1], F32, space="PSUM")
            nc.tensor.matmul(z3_ps[:], lhsT=z2T[:], rhs=wm3[:], start=True, stop=False)
            nc.tensor.matmul(
                z3_ps[:], lhsT=onesM[:].to_broadcast([1, 1]), rhs=bm3[:],
                start=False, stop=True,
            )
            z3 = mpool.tile([1, 1], F32)
            nc.vector.tensor_copy(z3[:], z3_ps[:])
            nc.sync.dma_start(out=z_out[:, :], in_=z3[:])

    _split_waits(nc)
    return nc


def make_in_maps(prep, inputs):
    W1 = np.asarray(inputs["W1"], np.float32)
    b1 = np.asarray(inputs["b1"], np.float32)
    att = np.asarray(inputs["att"], np.float32)
    Wpin = np.asarray(inputs["Wpin"], np.float32)
    Wpost = np.asarray(inputs["Wpost"], np.float32)
    WR = np.vstack([W1, Wpin, b1[None, :]]).astype(np.float32)  # (37, 64)
    in_maps = []
    for c in range(NCORES):
        pc = prep["per_core"][c]
        idx_map = {}
        for ch in range(prep["nchC"]):
            idx_map[f"i1_{ch}"] = pc["i1"][ch]
            idx_map[f"i2_{ch}"] = pc["i2"][ch]
        in_maps.append(
            dict(
                hpf=pc["hpf_b"],
                **idx_map,
                hn=pc["hn_b"],
                mrep=pc["mrep_b"],
                WR=WR,
                WRT=np.ascontiguousarray(WR.T),
                att2=att[C:, None].copy(),
                att1=att[:C, None].copy(),
                W1bT=np.ascontiguousarray(np.vstack([W1, b1[None, :]]).T),
                Wpost=Wpost.astype(np.float32),
                bpostT=np.asarray(inputs["bpost"], np.float32)[:, None],
                Wm1=np.asarray(inputs["Wm1"], np.float32),
                bm1=np.asarray(inputs["bm1"], np.float32)[None, :],
                Wm2=np.asarray(inputs["Wm2"], np.float32),
                bm2=np.asarray(inputs["bm2"], np.float32)[None, :],
                Wm3=np.asarray(inputs["Wm3"], np.float32),
                bm3=np.asarray(inputs["bm3"], np.float32)[None, :],
            )
        )
    return in_maps


# ----------------------------------------------------------- numpy emulator
def lrelu_np(v):
    return np.where(v >= 0, v, SLOPE * v)


def emulate(inputs, prep=None):
    """Mirror the device program with numpy (fp32; bf16 rounding on tables)."""
    if prep is None:
        prep = preprocess(inputs)
    NRE, TN, TE = prep["NRE"], prep["TN"], prep["TE"]
    W1 = np.asarray(inputs["W1"], np.float32)
    b1 = np.asarray(inputs["b1"], np.float32)
    att = np.asarray(inputs["att"], np.float32)
    Wpin = np.asarray(inputs["Wpin"], np.float32)
    WR = np.vstack([W1, Wpin, b1[None, :]])  # (37, 64)
    R = np.hstack([WR, (WR @ att[C:])[:, None]])  # (37, 65)
    w_u = np.vstack([W1, b1[None, :]]) @ att[:C]  # (33,)

    ef_full = np.zeros((NCORES * NRE, W_EF), np.float32)
    u_all = []
    for c in range(NCORES):
        pc = prep["per_core"][c]
        hpf = pc["hpf"]  # (nchA, P, K_CH, 36) fp32
        hn = _bf16_round(pc["hn"])  # (P, TN, 33)
        u_all.append((hn * _bf16_round(w_u)[None, None, :]).sum(-1))  # (P, TN)
        ef_shard = np.zeros((NRE, W_EF), np.float32)
        for (ch, col, d, tl) in prep["runsA"]:
            T = len(tl)
            seg = hpf[ch][:, col : col + T * d].reshape(P, T, d, F_HPF)
            hs = np.concatenate(
                [seg.sum(2), np.full((P, T, 1), float(d), np.float32)], -1
            ) * (1.0 / d)  # (P, T, 37)
            ef = hs @ R  # (P, T, 65)
            t0 = tl[0]
            for ti in range(T):
                rows = slice((t0 + ti) * P, (t0 + ti + 1) * P)
                ef_shard[rows, : C + 1] = _bf16_round(ef[:, ti])
        ef_full[c * NRE : (c + 1) * NRE] = ef_shard

    Wpost = np.asarray(inputs["Wpost"], np.float32)
    bpost = np.asarray(inputs["bpost"], np.float32)
    pool_g = np.zeros(C, np.float32)
    pool_m = np.zeros(C, np.float32)
    for c in range(NCORES):
        pc = prep["per_core"][c]
        cidx = pc["cidx"]
        u = u_all[c]
        multn = pc["multn"]
        for (ch, col, d, tl) in prep["runsC"]:
            T = len(tl)
            t0 = tl[0]
            stg = ef_full[cidx[ch]]  # (P, K_CH, W_EF)
            seg = stg[:, col : col + T * d].reshape(P, T, d, W_EF)
            eg = seg[..., :C]
            v = seg[..., C]
            uu = u[:, t0 : t0 + T]
            a = lrelu_np(uu[:, :, None] + v)
            ex = np.exp(a)
            den = ex.sum(2)
            alpha = _bf16_round(ex / den[:, :, None])
            msg = eg * alpha[..., None]
            mean = msg.sum(2) / d
            mx = msg.max(2)
            mn = msg.min(2)
            sq = (msg.astype(np.float32) * msg).sum(2) / d
            std = np.sqrt(np.maximum(sq - mean * mean, 0.0) + 1e-12)
            pna = np.concatenate([mean, mx, mn, std], -1)  # (P, T, 4C)
            hx = lrelu_np(pna @ Wpost + bpost)  # (P,T,C)
            for ti in range(T):
                pool_g += hx[:, ti].sum(0)
                pool_m += (multn[:, t0 + ti : t0 + ti + 1] * hx[:, ti]).sum(0)

    pooled = np.concatenate([pool_m / N_MACRO, pool_g / N_NODES])
    z = lrelu_np(pooled @ inputs["Wm1"] + inputs["bm1"])
    z = lrelu_np(z @ inputs["Wm2"] + inputs["bm2"])
    return (z @ inputs["Wm3"] + inputs["bm3"])[None, :]


def _install_ntff_hook():
    import sys
    import types

    try:
        if "antenv.axon_hooks" not in sys.modules:
            import antenv

            mod = types.ModuleType("antenv.axon_hooks")
            holder = [None]
            mod.set_axon_ntff_profile_hook = lambda h: holder.__setitem__(0, h)
            mod.get_axon_ntff_profile_hook = lambda: holder[0]
            mod._holder = holder
            sys.modules["antenv.axon_hooks"] = mod
            antenv.axon_hooks = mod
        mod = sys.modules["antenv.axon_hooks"]
        if mod.get_axon_ntff_profile_hook() is None:
            from trn_agent_boot.trn_boot import _ntff_profile_via_ctypes

            mod.set_axon_ntff_profile_hook(
                _ntff_profile_via_ctypes("/opt/axon/libaxon_pjrt.so")
            )
        return mod.get_axon_ntff_profile_hook() is not None
    except Exception:
        return False


_LAST = {}


def kernel(**inputs):
    prep = preprocess(inputs)
    nc = build_program(prep)
    in_maps = make_in_maps(prep, inputs)
    trace_ok = _install_ntff_hook()
    try:
        res = run_bass_kernel_spmd(
            nc, in_maps, list(range(NCORES)), trace=trace_ok, trace_cores=[0]
        )
    except Exception:
        res = run_bass_kernel_spmd(nc, in_maps, list(range(NCORES)))
    _LAST["res"] = res
    return res.results[0]["z"].astype(np.float32)



# revision 23
# speedup vs baseline: 1.8912x; 1.8912x over previous
"""Trainium2 Bass kernel for nn_BNet (hypergraph GNN message passing), 8 cores.

V2 design (vs baseline):
- Phase A has NO device gather: h[node]|pin_feature is staged host-side per
  pin slot (hpf, bf16, hyperedge-major degree-grouped layout, the same class
  of index-driven staging the baseline uses for pin_feature/aidx). One strided
  DVE reduce per run gives per-hyperedge sums; e_feat+v come from one PE
  transpose + one matmul per tile against R = [[W1],[Wpin],[b1] | .@att2]
  built on device. This removes the xl table, its AllGather, and 1310 of the
  baseline's 2396 serialized INDIRECT1D instructions.
- ef table is bf16 66-wide (64 e_feat + v + pad), so the phase C gather moves
  132B/row instead of 260B, and the AllGather shrinks 2x.
- Phase C keeps per-column indirect_dma_start (a 122k-row table cannot use
  the batched int16 dma_gather), ~1086 instructions serialized on GpSimd.
- Attention softmax without max-subtraction (exp args are O(5); tol is 2e-2).
- Phase C tail: PNA aggregates per tile, PE-transposed to pnaT (bf16), then
  hxT = Wpost^T @ pnaT batched over <=4 tiles per matmul pair; global pool =
  DVE column-reduce of hxT; macro pool via host-staged multiplicity rows on
  the few blocks containing macro nodes (macro nodes sort first in their
  degree class). AllReduce + tiny MLP head as baseline.
- u (node attention logit) from a host-staged node-tile h table via DVE.
"""

import numpy as np

import bass_rust
import concourse.bass as bass
import concourse.tile as tile
from concourse import mybir
from concourse.bass_utils import run_bass_kernel_spmd
from concourse.masks import make_identity
from concourse.vector_clock import ScopedClock

try:
    import ml_dtypes

    BF16_NP = np.dtype(ml_dtypes.bfloat16)
except Exception:  # pragma: no cover
    BF16_NP = None

# ----------------------------------------------------------------- constants
N_NODES = 200000
N_HE = 100000
NNZ = 1000000
F_H = 32  # 29 + 2 + 1
F_HPF = 36  # h + pin_feature
C = 64
NCORES = 8
P = 128
K_CH = 128  # slots per partition per chunk
MAX_RUN = 8
W_EF = 66  # ef table row: 64 e_feat + v + pad (bf16, 132B)
SLOPE = 0.1
N_MACRO = 512
F32 = mybir.dt.float32
BF16 = mybir.dt.bfloat16
I32 = mybir.dt.int32
AX = mybir.AxisListType.X
AF = mybir.ActivationFunctionType


# ------------------------------------------------------- walrus workarounds
def _patched_drain_and_barrier(self, tick_clock, wait_clock):
    nc = self.nc
    assert self.sems is not None
    handles = list(self.sems.allocated().values())
    scratch = nc.sync.sem_inc(handles[0], 0) if handles else nc.sync.drain()
    wait_clock.add_sem_waits(scratch.ins, ScopedClock({None: tick_clock.global_clock}))
    waits = list(scratch.ins.sync_info.on_wait)
    scratch.ins.sync_info = bass_rust.SyncInfo(on_wait=[], on_update=[])
    by_name = {h.name: h for h in handles}
    for w in waits:
        nc.sync.wait_ge(by_name[w.ant_name], w.wait_value)
    nc.sync.drain()
    nc.all_engine_barrier()
    popped = nc._tile_sem_poison_stack.pop()
    assert popped is self._sem_poison
    nc.clear_and_free_semaphores(handles)
    nc.all_engine_barrier()


tile.TileContext._drain_and_barrier = _patched_drain_and_barrier

_WS_CTR = [0]


def _split_waits(nc):
    """This walrus build allows at most one sync-wait per instruction; hoist
    extras onto NoOps inserted just before, same engine."""
    for fn in nc.m.functions:
        for bb in fn.blocks:
            insts = list(bb.instructions)
            new = []
            for inst in insts:
                si = inst.sync_info
                if si is not None and len(si.on_wait) > 1:
                    waits = list(si.on_wait)
                    for w in waits[:-1]:
                        _WS_CTR[0] += 1
                        new.append(
                            mybir.InstNoOp(
                                name=f"waitsplit_{_WS_CTR[0]}",
                                engine=inst.engine,
                                sync_info=mybir.SyncInfo(on_wait=[w], on_update=[]),
                                bass_nofuse=True,
                            )
                        )
                    inst.sync_info = mybir.SyncInfo(
                        on_wait=[waits[-1]], on_update=list(si.on_update)
                    )
                new.append(inst)
            bb.instructions = new


def _to_bf16(a):
    a = np.ascontiguousarray(a, dtype=np.float32)
    if BF16_NP is not None:
        return a.astype(BF16_NP)
    u = a.view(np.uint32)
    return ((u + 0x8000) >> 16).astype(np.uint16)


def _bf16_round(a):
    """fp32 -> nearest bf16 -> fp32 (for the numpy emulator)."""
    a = np.ascontiguousarray(a, dtype=np.float32)
    if BF16_NP is not None:
        return a.astype(BF16_NP).astype(np.float32)
    u = a.view(np.uint32)
    r = ((u + 0x8000) & 0xFFFF0000).astype(np.uint32)
    return r.view(np.float32)


# ----------------------------------------------------------- preprocessing
def _partition_by_degree(deg, ncores, first_flag=None):
    """Deal ids with deg>=1 round-robin per degree class across cores.
    first_flag: optional bool array - ids with True sort first in their class.
    """
    n = len(deg)
    if first_flag is None:
        order = np.lexsort((np.arange(n), deg))
    else:
        order = np.lexsort((np.arange(n), (~first_flag).astype(np.int8), deg))
    order = order[deg[order] >= 1]
    d_sorted = deg[order].astype(np.int64)
    change = np.nonzero(np.diff(d_sorted))[0] + 1
    starts = np.r_[0, change]
    ends = np.r_[change, len(order)]
    rank = np.arange(len(order)) - np.repeat(starts, ends - starts)
    core_of = (rank % ncores).astype(np.int32)
    lrank = rank // ncores
    tiles = []
    local = np.zeros(len(order), np.int64)
    base = 0
    for s, e in zip(starts, ends):
        d = int(d_sorted[s])
        m = int(np.ceil((e - s) / ncores))
        t_d = int(np.ceil(m / P))
        idx = slice(s, e)
        local[idx] = base + lrank[idx]
        for t in range(t_d):
            tiles.append((d, base + t * P))
        base += t_d * P
    n_rows = base + P  # final all-dummy tile (sentinel rows)
    core = np.full(n, -1, np.int32)
    loc = np.full(n, -1, np.int64)
    core[order] = core_of
    loc[order] = local
    return core, loc, tiles, n_rows


def _pack_chunks(tiles):
    place = []
    chunk, cur = 0, 0
    used = {}
    for d, _ in tiles:
        if cur + d > K_CH:
            chunk += 1
            cur = 0
        place.append((chunk, cur))
        cur += d
        used[chunk] = cur
    n_chunks = chunk + 1
    runs = []
    i = 0
    while i < len(tiles):
        d = tiles[i][0]
        ch, col = place[i]
        j = i
        while (
            j + 1 < len(tiles)
            and tiles[j + 1][0] == d
            and place[j + 1][0] == ch
            and j + 1 - i + 1 <= MAX_RUN
        ):
            j += 1
        runs.append((ch, col, d, list(range(i, j + 1))))
        i = j + 1
    return place, n_chunks, runs, used


def _rank_within(seg_ids):
    order = np.argsort(seg_ids, kind="stable")
    sorted_ids = seg_ids[order]
    change = np.nonzero(np.diff(sorted_ids))[0] + 1
    starts = np.r_[0, change]
    counts = np.diff(np.r_[starts, len(sorted_ids)])
    r = np.arange(len(sorted_ids)) - np.repeat(starts, counts)
    out = np.empty(len(seg_ids), np.int64)
    out[order] = r
    return out


def _blocks_of(runs):
    """Split runs' tiles into groups of <=4 for the hxT matmul blocks.
    Returns [(ch, d, t0, n_tiles, col_of_group)] in device order."""
    blocks = []
    for (ch, col, d, tl) in runs:
        for g0 in range(0, len(tl), 4):
            g = tl[g0 : g0 + 4]
            blocks.append((ch, d, g[0], len(g), col + g0 * d))
    return blocks


def preprocess(inputs):
    x = np.asarray(inputs["x"], np.float32)
    fake_pos = np.asarray(inputs["fake_pos"], np.float32)
    edge_index = np.asarray(inputs["edge_index"])
    pin_feature = np.asarray(inputs["pin_feature"], np.float32)
    macro_index = np.asarray(inputs["macro_index"])
    node_idx = edge_index[0].astype(np.int64)
    he_idx = edge_index[1].astype(np.int64)

    deg_n = np.bincount(node_idx, minlength=N_NODES)
    deg_e = np.bincount(he_idx, minlength=N_HE)

    ismacro = np.zeros(N_NODES, np.float32)
    ismacro[macro_index] = 1.0
    mult = np.bincount(macro_index, minlength=N_NODES).astype(np.float32)

    core_n, loc_n, tiles_n, NRN = _partition_by_degree(
        deg_n, NCORES, first_flag=mult > 0
    )
    core_e, loc_e, tiles_e, NRE = _partition_by_degree(deg_e, NCORES)
    placeA, nchA, runsA, usedA = _pack_chunks(tiles_e)
    placeC, nchC, runsC, usedC = _pack_chunks(tiles_n)
    TN = len(tiles_n)
    TE = len(tiles_e)

    erow = core_e.astype(np.int64) * NRE + loc_e  # ef_full row per hyperedge
    sent_g = TE * P  # first dummy-tile row of shard 0 (zeroed on device)

    h_full = np.concatenate([x, fake_pos, ismacro[:, None]], 1)  # (N, 32)

    # --- per-pin placement, hyperedge-major (phase A) ---
    jA = _rank_within(he_idx)
    cA = core_e[he_idx]
    tA = loc_e[he_idx] // P
    pA = loc_e[he_idx] % P
    chA = np.array([pl[0] for pl in placeA], np.int64)[tA]
    colA = np.array([pl[1] for pl in placeA], np.int64)[tA] + jA

    # --- node-major (phase C) ---
    jC = _rank_within(node_idx)
    cC = core_n[node_idx]
    tC = loc_n[node_idx] // P
    pC = loc_n[node_idx] % P
    chC = np.array([pl[0] for pl in placeC], np.int64)[tC]
    colC = np.array([pl[1] for pl in placeC], np.int64)[tC] + jC

    blocksC = _blocks_of(runsC)

    per_core = []
    for c in range(NCORES):
        # ---- phase A staging: hpf [nchA, P, K_CH, 36] bf16
        hpf = np.zeros((nchA, P, K_CH, F_HPF), np.float32)
        m = cA == c
        hpf[chA[m], pA[m], colA[m], :F_H] = h_full[node_idx[m]]
        hpf[chA[m], pA[m], colA[m], F_H:] = pin_feature[m]

        # ---- phase C staging: int32 ef_full row per slot
        m2 = cC == c
        cidx = np.full((nchC, P, K_CH), sent_g, np.int32)
        cidx[chC[m2], pC[m2], colC[m2]] = erow[he_idx[m2]].astype(np.int32)

        # ---- hn: node-tile h table [P, TN, 33] (col 32 = 1 for real rows)
        hn = np.zeros((P, TN, F_H + 1), np.float32)
        sel = core_n == c
        nl = loc_n[sel]
        hn[nl % P, nl // P, :F_H] = h_full[sel]
        hn[nl % P, nl // P, F_H] = 1.0

        # ---- macro multiplicity rows per hxT block
        multn = np.zeros((P, TN), np.float32)
        multn[nl % P, nl // P] = mult[sel]
        mrep_blocks = {}
        for bi, (ch, d, t0, G, col) in enumerate(blocksC):
            w = np.concatenate([multn[:, t0 + ti] for ti in range(G)])
            if w.any():
                mrep_blocks[bi] = w

        per_core.append(
            dict(
                hpf=hpf,
                cidx=cidx,
                hn=hn,
                mrep_blocks=mrep_blocks,
                multn=multn,
            )
        )

    # union of macro blocks across cores (device program is shared)
    mblk_union = {}
    off = 0
    for bi in sorted(set().union(*[set(pc["mrep_blocks"]) for pc in per_core])):
        G = blocksC[bi][3]
        mblk_union[bi] = (off, G * P)
        off += G * P
    MRC = max(off, P)
    for pc in per_core:
        mrep = np.zeros((C, MRC), np.float32)
        for bi, w in pc["mrep_blocks"].items():
            o, n = mblk_union[bi]
            mrep[:, o : o + len(w)] = w[None, :]
        pc["mrep"] = mrep
        pc["hpf_b"] = _to_bf16(pc["hpf"].reshape(nchA, P, K_CH * F_HPF))
        pc["hn_b"] = _to_bf16(pc["hn"].reshape(P, TN * (F_H + 1)))
        pc["mrep_b"] = _to_bf16(mrep)

    return dict(
        per_core=per_core,
        tiles_n=tiles_n,
        tiles_e=tiles_e,
        runsA=runsA,
        runsC=runsC,
        usedA=usedA,
        usedC=usedC,
        nchA=nchA,
        nchC=nchC,
        NRN=NRN,
        NRE=NRE,
        TN=TN,
        TE=TE,
        sent_g=sent_g,
        blocksC=blocksC,
        mblk_union=mblk_union,
        MRC=MRC,
        core_n=core_n,
        loc_n=loc_n,
        core_e=core_e,
        loc_e=loc_e,
        erow=erow,
    )


def _const_col_factory(nc, pool):
    cache = {}

    def cc(val, parts=P):
        key = (float(val), parts)
        if key not in cache:
            t = pool.tile([parts, 1], F32, tag=f"cc{len(cache)}")
            nc.vector.memset(t[:], float(val))
            cache[key] = t
        return cache[key][:]

    return cc


# ----------------------------------------------------------- device program
def build_program(prep, debug=False):
    NRE = prep["NRE"]
    nchA, nchC = prep["nchA"], prep["nchC"]
    runsA, runsC = prep["runsA"], prep["runsC"]
    TN, TE = prep["TN"], prep["TE"]
    blocksC = prep["blocksC"]
    mblk_union = prep["mblk_union"]
    MRC = prep["MRC"]
    core_ids = list(range(NCORES))

    nc = bass.Bass("TRN2", target_bir_lowering=False, debug=False, num_devices=NCORES)

    # ---- inputs
    hpf_in = nc.declare_dram_parameter("hpf", [nchA, P, K_CH * F_HPF], BF16, isOutput=False)
    cidx_in = nc.declare_dram_parameter("cidx", [nchC, P, K_CH], I32, isOutput=False)
    hn_in = nc.declare_dram_parameter("hn", [P, TN * (F_H + 1)], BF16, isOutput=False)
    mrep_in = nc.declare_dram_parameter("mrep", [C, MRC], BF16, isOutput=False)
    WR_in = nc.declare_dram_parameter("WR", [F_HPF + 1, C], F32, isOutput=False)
    WRT_in = nc.declare_dram_parameter("WRT", [C, F_HPF + 1], F32, isOutput=False)
    att2_in = nc.declare_dram_parameter("att2", [C, 1], F32, isOutput=False)
    att1_in = nc.declare_dram_parameter("att1", [C, 1], F32, isOutput=False)
    W1bT_in = nc.declare_dram_parameter("W1bT", [C, F_H + 1], F32, isOutput=False)
    Wpost_in = nc.declare_dram_parameter("Wpost", [4 * C, C], F32, isOutput=False)
    bpostT_in = nc.declare_dram_parameter("bpostT", [C, 1], F32, isOutput=False)
    Wm1_in = nc.declare_dram_parameter("Wm1", [2 * C, C], F32, isOutput=False)
    bm1_in = nc.declare_dram_parameter("bm1", [1, C], F32, isOutput=False)
    Wm2_in = nc.declare_dram_parameter("Wm2", [C, C // 2], F32, isOutput=False)
    bm2_in = nc.declare_dram_parameter("bm2", [1, C // 2], F32, isOutput=False)
    Wm3_in = nc.declare_dram_parameter("Wm3", [C // 2, 1], F32, isOutput=False)
    bm3_in = nc.declare_dram_parameter("bm3", [1, 1], F32, isOutput=False)
    z_out = nc.declare_dram_parameter("z", [1, 1], F32, isOutput=True)
    if debug:
        ef_dbg = nc.declare_dram_parameter("ef_dbg", [NRE, W_EF], BF16, isOutput=True)
        u_dbg = nc.declare_dram_parameter("u_dbg", [P, TN], F32, isOutput=True)
        pool_dbg = nc.declare_dram_parameter("pool_dbg", [2, C], F32, isOutput=True)

    # ---- internal DRAM
    ef_shard = nc.dram_tensor("ef_shard", [NRE, W_EF], BF16)
    ef_full = nc.dram_tensor("ef_full", [NCORES * NRE, W_EF], BF16, addr_space="Shared")
    ar_in = nc.dram_tensor("ar_in", [2, C], F32)
    ar_out = nc.dram_tensor("ar_out", [2, C], F32, addr_space="Shared")

    # persistent SBUF (used across contexts)
    u_sb = nc.alloc_sbuf_tensor("u_sb", [P, TN], F32)

    # =========== context 1: consts + u + phase A (e_feat) ===========
    with tile.TileContext(nc) as tc:
        with (
            tc.tile_pool(name="acons", bufs=1) as acons,
            tc.tile_pool(name="ahpf", bufs=3) as ahpf,
            tc.tile_pool(name="arun", bufs=3) as arun,
            tc.tile_pool(name="apsum", bufs=1, space="PSUM") as apsum,
            tc.tile_pool(name="apsum2", bufs=2, space="PSUM") as apsum2,
        ):
            ident = acons.tile([P, P], F32)
            make_identity(nc, ident[:])
            ccA = _const_col_factory(nc, acons)

            # R = [[W1],[Wpin],[b1]] with 65th column = R @ att2
            R = acons.tile([F_HPF + 1, C + 1], F32)
            nc.sync.dma_start(out=R[:, :C], in_=WR_in[:, :])
            wrt = acons.tile([C, F_HPF + 1], F32)
            nc.sync.dma_start(out=wrt[:], in_=WRT_in[:, :])
            att2 = acons.tile([C, 1], F32)
            nc.sync.dma_start(out=att2[:], in_=att2_in[:, :])
            cps = apsum.tile([P, P], F32, space="PSUM", tag="cps")
            nc.tensor.matmul(
                cps[: F_HPF + 1, :1], lhsT=wrt[:], rhs=att2[:], start=True, stop=True
            )
            nc.scalar.activation(R[:, C : C + 1], cps[: F_HPF + 1, :1], AF.Copy)

            # w_u broadcast to all partitions: wur[p, f] = (W1b @ att1)[f]
            w1bt = acons.tile([C, F_H + 1], F32)
            nc.sync.dma_start(out=w1bt[:], in_=W1bT_in[:, :])
            att1 = acons.tile([C, 1], F32)
            nc.sync.dma_start(out=att1[:], in_=att1_in[:, :])
            cps2 = apsum.tile([P, P], F32, space="PSUM", tag="cps")
            nc.tensor.matmul(
                cps2[: F_H + 1, :1], lhsT=w1bt[:], rhs=att1[:], start=True, stop=True
            )
            wu = acons.tile([F_H + 1, 1], F32)
            nc.vector.tensor_copy(wu[:], cps2[: F_H + 1, :1])
            cps3 = apsum.tile([P, P], F32, space="PSUM", tag="cps")
            nc.tensor.transpose(
                out=cps3[:1, : F_H + 1], in_=wu[:], identity=ident[: F_H + 1, : F_H + 1]
            )
            wuT = acons.tile([1, F_H + 1], F32)
            nc.vector.tensor_copy(wuT[:], cps3[:1, : F_H + 1])
            ones1 = acons.tile([1, P], F32)
            nc.vector.memset(ones1[:], 1.0)
            cps4 = apsum.tile([P, P], F32, space="PSUM", tag="cps")
            nc.tensor.matmul(
                cps4[:, : F_H + 1], lhsT=ones1[:], rhs=wuT[:], start=True, stop=True
            )
            wur = acons.tile([P, F_H + 1], BF16)
            nc.scalar.activation(wur[:], cps4[:, : F_H + 1], AF.Copy)

            # u = reduce(hn * w_u_rep) over f, in tile-blocks
            hn_sb = acons.tile([P, TN * (F_H + 1)], BF16)
            nc.sync.dma_start(out=hn_sb[:], in_=hn_in[:, :])
            UB = 32
            for b0 in range(0, TN, UB):
                nb = min(UB, TN - b0)
                tmp = arun.tile([P, UB * (F_H + 1)], F32, tag="utmp")
                nc.vector.tensor_tensor(
                    out=tmp[:, : nb * (F_H + 1)].rearrange("p (t f) -> p t f", t=nb),
                    in0=hn_sb[:, b0 * (F_H + 1) : (b0 + nb) * (F_H + 1)].rearrange(
                        "p (t f) -> p t f", t=nb
                    ),
                    in1=wur[:, None, :].to_broadcast([P, nb, F_H + 1]),
                    op=mybir.AluOpType.mult,
                )
                nc.vector.reduce_sum(
                    u_sb[:, b0 : b0 + nb],
                    tmp[:, : nb * (F_H + 1)].rearrange("p (t f) -> p t f", t=nb),
                    axis=AX,
                )

            # zero the sentinel (dummy) tile rows of ef_shard
            zt = acons.tile([P, W_EF], BF16)
            nc.vector.memset(zt[:], 0.0)
            nc.sync.dma_start(out=ef_shard[TE * P : (TE + 1) * P, :], in_=zt[:])

            # ---- phase A main loop
            run_by_chunkA = {}
            for r in runsA:
                run_by_chunkA.setdefault(r[0], []).append(r)

            for ch in range(nchA):
                hpf_t = ahpf.tile([P, K_CH * F_HPF], BF16)
                nc.sync.dma_start(out=hpf_t[:], in_=hpf_in[ch])
                for (_, col, d, tl) in run_by_chunkA.get(ch, []):
                    T = len(tl)
                    t0 = tl[0]
                    FW = F_HPF + 1
                    hs = arun.tile([P, MAX_RUN * FW], F32, tag="hs")
                    hs3 = hs[:, : T * FW].rearrange("p (t f) -> p t f", t=T)
                    nc.vector.reduce_sum(
                        hs3[:, :, :F_HPF],
                        hpf_t[:, col * F_HPF : (col + T * d) * F_HPF].rearrange(
                            "p (t j f) -> p t f j", t=T, j=d
                        ),
                        axis=AX,
                    )
                    nc.vector.memset(hs3[:, :, F_HPF], float(d))
                    nc.scalar.activation(
                        hs[:, : T * FW], hs[:, : T * FW], AF.Copy, scale=ccA(1.0 / d)
                    )
                    efb = arun.tile([P, MAX_RUN * (C + 1)], BF16, tag="efb")
                    for ti in range(T):
                        hsT_ps = apsum2.tile([P, P], F32, space="PSUM", tag="hsT")
                        nc.tensor.transpose(
                            out=hsT_ps[:FW, :],
                            in_=hs[:, ti * FW : (ti + 1) * FW],
                            identity=ident[:],
                        )
                        hsT = arun.tile([FW, P], F32, tag="hsTs")
                        nc.scalar.activation(hsT[:], hsT_ps[:FW, :], AF.Copy)
                        ef_ps = apsum2.tile([P, C + 1], F32, space="PSUM", tag="efps")
                        nc.tensor.matmul(
                            ef_ps[:], lhsT=hsT[:], rhs=R[:], start=True, stop=True
                        )
                        nc.scalar.activation(
                            efb[:, ti * (C + 1) : (ti + 1) * (C + 1)], ef_ps[:], AF.Copy
                        )
                    nc.sync.dma_start(
                        out=ef_shard[t0 * P : (t0 + T) * P, : C + 1].rearrange(
                            "(t p) w -> p t w", p=P
                        ),
                        in_=efb[:, : T * (C + 1)].rearrange("p (t w) -> p t w", t=T),
                    )

    # =========== AllGather ef ===========
    with (
        nc.semaphore("ag_sem") as ag_sem,
        nc.Block() as blk,
    ):

        @blk.gpsimd
        def _(g):
            g.collective_compute(
                "AllGather",
                mybir.AluOpType.bypass,
                replica_groups=[core_ids],
                ins=[ef_shard[:, :]],
                outs=[ef_full[:, :]],
            ).then_inc(ag_sem, 1)
            g.wait_ge(ag_sem, 1)

    # =========== context 2: phase C ===========
    NBLK = len(blocksC)
    with tile.TileContext(nc) as tc:
        with (
            tc.tile_pool(name="ccons", bufs=1) as ccons,
            tc.tile_pool(name="cidxp", bufs=3) as cidxp,
            tc.tile_pool(name="cstg", bufs=3) as cstg,
            tc.tile_pool(name="cmsg", bufs=1) as cmsg,
            tc.tile_pool(name="csml", bufs=2) as csml,
            tc.tile_pool(name="cpost", bufs=2) as cpost,
            tc.tile_pool(name="cpT", bufs=2) as cpT,
            tc.tile_pool(name="chxp", bufs=2) as chxp,
            tc.tile_pool(name="ctpsum", bufs=4, space="PSUM") as ctpsum,
            tc.tile_pool(name="chpsum", bufs=1, space="PSUM") as chpsum,
        ):
            identC = ccons.tile([P, P], F32)
            make_identity(nc, identC[:])
            ccC = _const_col_factory(nc, ccons)
            wpost_a = ccons.tile([P, C], F32)
            nc.sync.dma_start(out=wpost_a[:], in_=Wpost_in[:P, :])
            wpost_b = ccons.tile([P, C], F32)
            nc.sync.dma_start(out=wpost_b[:], in_=Wpost_in[P:, :])
            bpostT = ccons.tile([C, 1], F32)
            nc.sync.dma_start(out=bpostT[:], in_=bpostT_in[:, :])
            mrep_sb = ccons.tile([C, MRC], BF16)
            nc.sync.dma_start(out=mrep_sb[:], in_=mrep_in[:, :])
            poolg = ccons.tile([C, NBLK + 1], F32)
            poolm = ccons.tile([C, NBLK + 1], F32)
            nc.vector.memset(poolg[:], 0.0)
            nc.vector.memset(poolm[:], 0.0)

            run_by_chunkC = {}
            for r in runsC:
                run_by_chunkC.setdefault(r[0], []).append(r)

            bi_ctr = [0]

            for ch in range(nchC):
                it = cidxp.tile([P, K_CH], I32)
                nc.sync.dma_start(out=it[:], in_=cidx_in[ch])
                stg = cstg.tile([P, K_CH * W_EF], BF16)
                for j in range(prep["usedC"][ch]):
                    nc.gpsimd.indirect_dma_start(
                        out=stg[:, j * W_EF : (j + 1) * W_EF],
                        out_offset=None,
                        in_=ef_full[:, :],
                        in_offset=bass.IndirectOffsetOnAxis(ap=it[:, j : j + 1], axis=0),
                    )

                for (_, col, d, tl) in run_by_chunkC.get(ch, []):
                    T = len(tl)
                    F = T * d
                    t0 = tl[0]
                    stg3 = stg[:, col * W_EF : (col + F) * W_EF].rearrange(
                        "p (s w) -> p s w", s=F
                    )
                    ef_ap = stg3[:, :, :C]
                    v_sc = stg3[:, :, C]

                    # vv = fp32 copy of v column; a = lrelu(u + v); ex = exp(a)
                    vv = csml.tile([P, K_CH], F32, tag="vv")
                    nc.vector.tensor_copy(vv[:, :F], v_sc)
                    asl = csml.tile([P, K_CH], F32, tag="asl")
                    nc.vector.tensor_tensor(
                        out=asl[:, :F].rearrange("p (t j) -> p t j", t=T),
                        in0=u_sb[:, t0 : t0 + T, None].to_broadcast([P, T, d]),
                        in1=vv[:, :F].rearrange("p (t j) -> p t j", t=T),
                        op=mybir.AluOpType.add,
                    )
                    a2 = csml.tile([P, K_CH], F32, tag="a2")
                    nc.vector.tensor_scalar_mul(a2[:, :F], asl[:, :F], SLOPE)
                    nc.vector.tensor_tensor(
                        out=asl[:, :F], in0=asl[:, :F], in1=a2[:, :F],
                        op=mybir.AluOpType.max,
                    )
                    ex = csml.tile([P, K_CH], F32, tag="ex")
                    nc.scalar.activation(ex[:, :F], asl[:, :F], AF.Exp)
                    den = csml.tile([P, MAX_RUN], F32, tag="den")
                    nc.vector.reduce_sum(
                        den[:, :T],
                        ex[:, :F].rearrange("p (t j) -> p t j", t=T),
                        axis=AX,
                    )
                    nc.vector.reciprocal(den[:, :T], den[:, :T])
                    alpha = csml.tile([P, K_CH], BF16, tag="alpha")
                    nc.vector.tensor_tensor(
                        out=alpha[:, :F].rearrange("p (t j) -> p t j", t=T),
                        in0=ex[:, :F].rearrange("p (t j) -> p t j", t=T),
                        in1=den[:, :T, None].to_broadcast([P, T, d]),
                        op=mybir.AluOpType.mult,
                    )
                    # msg = alpha * e_g (bf16)
                    msg = cmsg.tile([P, K_CH * C], F32, tag="msg")
                    nc.vector.tensor_tensor(
                        out=msg[:, : F * C].rearrange("p (s e) -> p s e", s=F),
                        in0=ef_ap,
                        in1=alpha[:, :F, None].to_broadcast([P, F, C]),
                        op=mybir.AluOpType.mult,
                    )
                    msg3 = msg[:, : F * C].rearrange(
                        "p (t j e) -> p t e j", t=T, j=d, e=C
                    )
                    post = cpost.tile([P, MAX_RUN * 4 * C], F32, tag="post")
                    post3 = post[:, : T * 4 * C].rearrange(
                        "p (t q e) -> p t q e", t=T, q=4
                    )
                    nc.vector.reduce_max(post3[:, :, 1], msg3, axis=AX)
                    nc.vector.tensor_reduce(
                        post3[:, :, 2], msg3, op=mybir.AluOpType.min, axis=AX
                    )
                    sm = cpost.tile([P, MAX_RUN * C], F32, tag="sm")
                    nc.vector.reduce_sum(
                        sm[:, : T * C].rearrange("p (t e) -> p t e", t=T), msg3, axis=AX
                    )
                    nc.scalar.activation(
                        post3[:, :, 0],
                        sm[:, : T * C].rearrange("p (t e) -> p t e", t=T),
                        AF.Copy,
                        scale=ccC(1.0 / d),
                    )
                    # msq = msg^2, then sumsq
                    msq = cmsg.tile([P, K_CH * C], F32, tag="msq")
                    nc.scalar.activation(msq[:, : F * C], msg[:, : F * C], AF.Square)
                    msq3 = msq[:, : F * C].rearrange(
                        "p (t j e) -> p t e j", t=T, j=d, e=C
                    )
                    sq = cpost.tile([P, MAX_RUN * C], F32, tag="sq")
                    nc.vector.reduce_sum(
                        sq[:, : T * C].rearrange("p (t e) -> p t e", t=T), msq3, axis=AX
                    )
                    m2 = cpost.tile([P, MAX_RUN * C], F32, tag="m2")
                    nc.vector.tensor_tensor(
                        out=m2[:, : T * C].rearrange("p (t e) -> p t e", t=T),
                        in0=post3[:, :, 0],
                        in1=post3[:, :, 0],
                        op=mybir.AluOpType.mult,
                    )
                    nc.vector.tensor_scalar(
                        out=sq[:, : T * C], in0=sq[:, : T * C],
                        scalar1=1.0 / d, scalar2=None, op0=mybir.AluOpType.mult,
                    )
                    nc.vector.tensor_tensor(
                        out=sq[:, : T * C], in0=sq[:, : T * C], in1=m2[:, : T * C],
                        op=mybir.AluOpType.subtract,
                    )
                    nc.vector.tensor_scalar_max(sq[:, : T * C], sq[:, : T * C], 0.0)
                    nc.scalar.activation(
                        post3[:, :, 3],
                        sq[:, : T * C].rearrange("p (t e) -> p t e", t=T),
                        AF.Sqrt,
                        bias=ccC(1e-12),
                    )

                    # hxT blocks of <=4 tiles
                    for g0 in range(0, T, 4):
                        G = min(4, T - g0)
                        bi = bi_ctr[0]
                        bi_ctr[0] += 1
                        assert blocksC[bi][2] == tl[g0], (bi, blocksC[bi], tl, g0)
                        pT = cpT.tile([P, 2 * 4 * P], F32, tag="pT")
                        for ti in range(G):
                            ta = g0 + ti
                            for half in range(2):
                                tp_ps = ctpsum.tile([P, P], F32, space="PSUM", tag="tp")
                                nc.tensor.transpose(
                                    out=tp_ps[:],
                                    in_=post[
                                        :,
                                        ta * 4 * C + half * P : ta * 4 * C
                                        + half * P
                                        + P,
                                    ],
                                    identity=identC[:],
                                )
                                nc.scalar.activation(
                                    pT[
                                        :,
                                        half * 4 * P + ti * P : half * 4 * P
                                        + (ti + 1) * P,
                                    ],
                                    tp_ps[:],
                                    AF.Copy,
                                )
                        hx_ps = chpsum.tile([C, 4 * P], F32, space="PSUM", tag="hx")
                        nc.tensor.matmul(
                            hx_ps[:, : G * P],
                            lhsT=wpost_a[:],
                            rhs=pT[:, : G * P],
                            start=True,
                            stop=False,
                        )
                        nc.tensor.matmul(
                            hx_ps[:, : G * P],
                            lhsT=wpost_b[:],
                            rhs=pT[:, 4 * P : 4 * P + G * P],
                            start=False,
                            stop=True,
                        )
                        hx = chxp.tile([C, 4 * P], F32, tag="hx")
                        nc.vector.tensor_tensor(
                            out=hx[:, : G * P],
                            in0=hx_ps[:, : G * P],
                            in1=bpostT[:].to_broadcast([C, G * P]),
                            op=mybir.AluOpType.add,
                        )
                        hx2 = chxp.tile([C, 4 * P], F32, tag="hx2")
                        nc.vector.tensor_scalar_mul(
                            hx2[:, : G * P], hx[:, : G * P], SLOPE
                        )
                        nc.vector.tensor_tensor(
                            out=hx[:, : G * P], in0=hx[:, : G * P],
                            in1=hx2[:, : G * P], op=mybir.AluOpType.max,
                        )
                        nc.vector.reduce_sum(
                            poolg[:, bi : bi + 1], hx[:, None, : G * P], axis=AX
                        )
                        if bi in mblk_union:
                            o, ncols = mblk_union[bi]
                            assert ncols == G * P
                            hxm = chxp.tile([C, 4 * P], F32, tag="hxm")
                            nc.vector.tensor_tensor(
                                out=hxm[:, : G * P],
                                in0=hx[:, : G * P],
                                in1=mrep_sb[:, o : o + G * P],
                                op=mybir.AluOpType.mult,
                            )
                            nc.vector.reduce_sum(
                                poolm[:, bi : bi + 1], hxm[:, None, : G * P], axis=AX
                            )

            assert bi_ctr[0] == NBLK
            # final pool reduction -> ar_in [2, C]
            pg = ccons.tile([C, 1], F32)
            nc.vector.reduce_sum(pg[:], poolg[:, None, :], axis=AX)
            pm = ccons.tile([C, 1], F32)
            nc.vector.reduce_sum(pm[:], poolm[:, None, :], axis=AX)
            pools2 = ccons.tile([C, 2], F32)
            nc.vector.tensor_copy(pools2[:, 0:1], pm[:])
            nc.vector.tensor_copy(pools2[:, 1:2], pg[:])
            poolsT_ps = chpsum.tile([2, C], F32, space="PSUM", tag="pt")
            nc.tensor.transpose(out=poolsT_ps[:], in_=pools2[:], identity=identC[:C, :C])
            poolsT = ccons.tile([2, C], F32)
            nc.vector.tensor_copy(poolsT[:], poolsT_ps[:])
            nc.sync.dma_start(out=ar_in[:, :], in_=poolsT[:])

    if debug:
        with (
            nc.semaphore("dbg_sem") as dbg_sem,
            nc.Block() as blkd,
        ):

            @blkd.gpsimd
            def _(g):
                g.dma_start(out=ef_dbg[:, :], in_=ef_shard[:, :]).then_inc(dbg_sem, 16)
                g.dma_start(out=pool_dbg[:, :], in_=ar_in[:, :]).then_inc(dbg_sem, 16)
                g.wait_ge(dbg_sem, 32)

        with tile.TileContext(nc) as tc:
            with tc.tile_pool(name="dbgp", bufs=1) as dbgp:
                ut = dbgp.tile([P, TN], F32)
                nc.vector.tensor_copy(ut[:], u_sb[:])
                nc.sync.dma_start(out=u_dbg[:, :], in_=ut[:])

    # =========== AllReduce pools ===========
    with (
        nc.semaphore("ar_sem") as ar_sem,
        nc.Block() as blk3,
    ):

        @blk3.gpsimd
        def _(g):
            g.collective_compute(
                "AllReduce",
                mybir.AluOpType.add,
                replica_groups=[core_ids],
                ins=[ar_in[:, :]],
                outs=[ar_out[:, :]],
            ).then_inc(ar_sem, 1)
            g.wait_ge(ar_sem, 1)

    # =========== context 3: MLP head ===========
    with tile.TileContext(nc) as tc:
        with (
            tc.tile_pool(name="mpool", bufs=1) as mpool,
            tc.tile_pool(name="mpsum", bufs=1, space="PSUM") as mpsum,
        ):
            identM = mpool.tile([P, P], F32)
            make_identity(nc, identM[:])
            onesM = mpool.tile([1, 1], F32)
            nc.vector.memset(onesM[:], 1.0)
            ccM = _const_col_factory(nc, mpool)
            pool2 = mpool.tile([2, C], F32)
            nc.sync.dma_start(out=pool2[:], in_=ar_out[:, :])
            poolT_ps = mpsum.tile([P, P], F32, space="PSUM")
            nc.tensor.transpose(out=poolT_ps[:C, :2], in_=pool2[:], identity=identM[:2, :2])
            pooled = mpool.tile([P, 1], F32)
            nc.scalar.activation(
                pooled[:C, :], poolT_ps[:C, :1], AF.Copy, scale=ccM(1.0 / N_MACRO, C)
            )
            nc.scalar.activation(
                pooled[C:, :], poolT_ps[:C, 1:2], AF.Copy, scale=ccM(1.0 / N_NODES, C)
            )
            wm1 = mpool.tile([2 * C, C], F32)
            nc.sync.dma_start(out=wm1[:], in_=Wm1_in[:, :])
            bm1 = mpool.tile([1, C], F32)
            nc.sync.dma_start(out=bm1[:], in_=bm1_in[:, :])
            wm2 = mpool.tile([C, C // 2], F32)
            nc.sync.dma_start(out=wm2[:], in_=Wm2_in[:, :])
            bm2 = mpool.tile([1, C // 2], F32)
            nc.sync.dma_start(out=bm2[:], in_=bm2_in[:, :])
            wm3 = mpool.tile([C // 2, 1], F32)
            nc.sync.dma_start(out=wm3[:], in_=Wm3_in[:, :])
            bm3 = mpool.tile([1, 1], F32)
            nc.sync.dma_start(out=bm3[:], in_=bm3_in[:, :])

            def _lrelu_row(dst, src_ps, width):
                tmp = mpool.tile([1, width], F32, tag=f"lr{width}")
                nc.scalar.activation(tmp[:], src_ps[:], AF.Copy, scale=ccM(SLOPE, 1))
                nc.vector.tensor_tensor(
                    out=dst[:], in0=src_ps[:], in1=tmp[:], op=mybir.AluOpType.max
                )

            z1_ps = mpsum.tile([1, C], F32, space="PSUM")
            nc.tensor.matmul(z1_ps[:], lhsT=pooled[:], rhs=wm1[:], start=True, stop=False)
            nc.tensor.matmul(
                z1_ps[:], lhsT=onesM[:].to_broadcast([1, 1]), rhs=bm1[:],
                start=False, stop=True,
            )
            z1 = mpool.tile([1, C], F32)
            _lrelu_row(z1, z1_ps, C)
            z1T_ps = mpsum.tile([P, P], F32, space="PSUM")
            nc.tensor.transpose(out=z1T_ps[:C, :1], in_=z1[:], identity=identM[:1, :1])
            z1T = mpool.tile([C, 1], F32)
            nc.vector.tensor_copy(z1T[:], z1T_ps[:C, :1])
            z2_ps = mpsum.tile([1, C // 2], F32, space="PSUM")
            nc.tensor.matmul(z2_ps[:], lhsT=z1T[:], rhs=wm2[:], start=True, stop=False)
            nc.tensor.matmul(
                z2_ps[:], lhsT=onesM[:].to_broadcast([1, 1]), rhs=bm2[:],
                start=False, stop=True,
            )
            z2 = mpool.tile([1, C // 2], F32)
            _lrelu_row(z2, z2_ps, C // 2)
            z2T_ps = mpsum.tile([P, P], F32, space="PSUM")
            nc.tensor.transpose(out=z2T_ps[: C // 2, :1], in_=z2[:], identity=identM[:1, :1])
            z2T = mpool.tile([C // 2, 1], F32)
            nc.vector.tensor_copy(z2T[:], z2T_ps[: C // 2, :1])
            z3_ps = mpsum.tile([1, 1], F32, space="PSUM")
            nc.tensor.matmul(z3_ps[:], lhsT=z2T[:], rhs=wm3[:], start=True, stop=False)
            nc.tensor.matmul(
                z3_ps[:], lhsT=onesM[:].to_broadcast([1, 1]), rhs=bm3[:],
                start=False, stop=True,
            )
            z3 = mpool.tile([1, 1], F32)
            nc.vector.tensor_copy(z3[:], z3_ps[:])
            nc.sync.dma_start(out=z_out[:, :], in_=z3[:])

    _split_waits(nc)
    return nc


def make_in_maps(prep, inputs):
    W1 = np.asarray(inputs["W1"], np.float32)
    b1 = np.asarray(inputs["b1"], np.float32)
    att = np.asarray(inputs["att"], np.float32)
    Wpin = np.asarray(inputs["Wpin"], np.float32)
    Wpost = np.asarray(inputs["Wpost"], np.float32)
    WR = np.vstack([W1, Wpin, b1[None, :]]).astype(np.float32)  # (37, 64)
    in_maps = []
    for c in range(NCORES):
        pc = prep["per_core"][c]
        in_maps.append(
            dict(
                hpf=pc["hpf_b"],
                cidx=pc["cidx"],
                hn=pc["hn_b"],
                mrep=pc["mrep_b"],
                WR=WR,
                WRT=np.ascontiguousarray(WR.T),
                att2=att[C:, None].copy(),
                att1=att[:C, None].copy(),
                W1bT=np.ascontiguousarray(np.vstack([W1, b1[None, :]]).T),
                Wpost=Wpost.astype(np.float32),
                bpostT=np.asarray(inputs["bpost"], np.float32)[:, None],
                Wm1=np.asarray(inputs["Wm1"], np.float32),
                bm1=np.asarray(inputs["bm1"], np.float32)[None, :],
                Wm2=np.asarray(inputs["Wm2"], np.float32),
                bm2=np.asarray(inputs["bm2"], np.float32)[None, :],
                Wm3=np.asarray(inputs["Wm3"], np.float32),
                bm3=np.asarray(inputs["bm3"], np.float32)[None, :],
            )
        )
    return in_maps


# ----------------------------------------------------------- numpy emulator
def lrelu_np(v):
    return np.where(v >= 0, v, SLOPE * v)


def emulate(inputs, prep=None):
    """Mirror the device program with numpy (fp32; bf16 rounding on tables)."""
    if prep is None:
        prep = preprocess(inputs)
    NRE, TN, TE = prep["NRE"], prep["TN"], prep["TE"]
    W1 = np.asarray(inputs["W1"], np.float32)
    b1 = np.asarray(inputs["b1"], np.float32)
    att = np.asarray(inputs["att"], np.float32)
    Wpin = np.asarray(inputs["Wpin"], np.float32)
    WR = np.vstack([W1, Wpin, b1[None, :]])  # (37, 64)
    R = np.hstack([WR, (WR @ att[C:])[:, None]])  # (37, 65)
    w_u = np.vstack([W1, b1[None, :]]) @ att[:C]  # (33,)

    ef_full = np.zeros((NCORES * NRE, W_EF), np.float32)
    u_all = []
    for c in range(NCORES):
        pc = prep["per_core"][c]
        hpf = pc["hpf"]  # (nchA, P, K_CH, 36) fp32
        hn = _bf16_round(pc["hn"])  # (P, TN, 33)
        u_all.append((hn * _bf16_round(w_u)[None, None, :]).sum(-1))  # (P, TN)
        ef_shard = np.zeros((NRE, W_EF), np.float32)
        for (ch, col, d, tl) in prep["runsA"]:
            T = len(tl)
            seg = hpf[ch][:, col : col + T * d].reshape(P, T, d, F_HPF)
            hs = np.concatenate(
                [seg.sum(2), np.full((P, T, 1), float(d), np.float32)], -1
            ) * (1.0 / d)  # (P, T, 37)
            ef = hs @ R  # (P, T, 65)
            t0 = tl[0]
            for ti in range(T):
                rows = slice((t0 + ti) * P, (t0 + ti + 1) * P)
                ef_shard[rows, : C + 1] = _bf16_round(ef[:, ti])
        ef_full[c * NRE : (c + 1) * NRE] = ef_shard

    Wpost = np.asarray(inputs["Wpost"], np.float32)
    bpost = np.asarray(inputs["bpost"], np.float32)
    pool_g = np.zeros(C, np.float32)
    pool_m = np.zeros(C, np.float32)
    for c in range(NCORES):
        pc = prep["per_core"][c]
        cidx = pc["cidx"]
        u = u_all[c]
        multn = pc["multn"]
        for (ch, col, d, tl) in prep["runsC"]:
            T = len(tl)
            t0 = tl[0]
            stg = ef_full[cidx[ch]]  # (P, K_CH, W_EF)
            seg = stg[:, col : col + T * d].reshape(P, T, d, W_EF)
            eg = seg[..., :C]
            v = seg[..., C]
            uu = u[:, t0 : t0 + T]
            a = lrelu_np(uu[:, :, None] + v)
            ex = np.exp(a)
            den = ex.sum(2)
            alpha = _bf16_round(ex / den[:, :, None])
            msg = eg * alpha[..., None]
            mean = msg.sum(2) / d
            mx = msg.max(2)
            mn = msg.min(2)
            sq = (msg.astype(np.float32) * msg).sum(2) / d
            std = np.sqrt(np.maximum(sq - mean * mean, 0.0) + 1e-12)
            pna = np.concatenate([mean, mx, mn, std], -1)  # (P, T, 4C)
            hx = lrelu_np(pna @ Wpost + bpost)  # (P,T,C)
            for ti in range(T):
                pool_g += hx[:, ti].sum(0)
                pool_m += (multn[:, t0 + ti : t0 + ti + 1] * hx[:, ti]).sum(0)

    pooled = np.concatenate([pool_m / N_MACRO, pool_g / N_NODES])
    z = lrelu_np(pooled @ inputs["Wm1"] + inputs["bm1"])
    z = lrelu_np(z @ inputs["Wm2"] + inputs["bm2"])
    return (z @ inputs["Wm3"] + inputs["bm3"])[None, :]


def _install_ntff_hook():
    import sys
    import types

    try:
        if "antenv.axon_hooks" not in sys.modules:
            import antenv

            mod = types.ModuleType("antenv.axon_hooks")
            holder = [None]
            mod.set_axon_ntff_profile_hook = lambda h: holder.__setitem__(0, h)
            mod.get_axon_ntff_profile_hook = lambda: holder[0]
            mod._holder = holder
            sys.modules["antenv.axon_hooks"] = mod
            antenv.axon_hooks = mod
        mod = sys.modules["antenv.axon_hooks"]
        if mod.get_axon_ntff_profile_hook() is None:
            from trn_agent_boot.trn_boot import _ntff_profile_via_ctypes

            mod.set_axon_ntff_profile_hook(
                _ntff_profile_via_ctypes("/opt/axon/libaxon_pjrt.so")
            )
        return mod.get_axon_ntff_profile_hook() is not None
    except Exception:
        return False


_LAST = {}


def kernel(**inputs):
    prep = preprocess(inputs)
    nc = build_program(prep)
    in_maps = make_in_maps(prep, inputs)
    trace_ok = _install_ntff_hook()
    try:
        res = run_bass_kernel_spmd(
            nc, in_maps, list(range(NCORES)), trace=trace_ok, trace_cores=[0]
        )
    except Exception:
        res = run_bass_kernel_spmd(nc, in_maps, list(range(NCORES)))
    _LAST["res"] = res
    return res.results[0]["z"].astype(np.float32)

